# revision 1
# baseline (speedup 1.0000x reference)
"""Trainium2 Bass kernel for causal multi-head attention (dense transformer block).

Math (reference semantics):
    qkv = x @ w_qkv.T ; split into Q,K,V heads [B,H,T,dk]
    (rotary in the reference rotates Q and K of head h by a constant,
     time-independent orthogonal rotation R_h; since scores = (R_h q)·(R_h k)
     = q·k, the rotation cancels exactly and is skipped here)
    scores = causal_mask(Q @ K.T / sqrt(dk)); attn = softmax(scores)
    out = attn @ V ; y = out @ w_o.T

Sharding: head-parallel over 8 cores (2 heads/core, both batches).  Each core
computes a partial y (its heads' contribution through w_o columns); the host
sums the 8 partials (the "all-reduce").

v3 design (vs the f32r baseline):
  * All matmul inputs bf16 (same 1 cyc/row PE rate as f32r at wide free dims,
    half the DMA + SBUF).  PSUM accumulation stays fp32.
  * Phase 1 is k-outer: 6 concurrent PSUM groups (Q/K for 2 heads in two
    2-bank "S" tiles, V written DIRECTLY in [token, dk] layout into "O"
    tiles) so PE starts as soon as the first weight/x chunk lands and no
    V^T->V transposes are needed.  PSUM drains on ACT/DVE (GpSimd cannot
    touch PSUM), chunk-PAIR DMAs halve HWDGE descriptor-queue pressure.
  * Softmax denominator: bf16 pair-add (DVE 4x mode) + fp32 running sums
    split into two chains (GpSimd + DVE), then two GpSimd cross-partition
    (axis=C) reduces - no [1,512] ones-matmuls on PE (saves ~34us PE).
  * Causal narrowing: diagonal key-tiles only compute the live q-suffix in
    scores/AV/exp; the dead ex prefix is zeroed by a GpSimd memset; the
    128x128 causal triangle is masked by a GpSimd multiply.
  * proj(qb) units are woven between the attention kt-pairs of the next
    unit (qb3 into the next batch's phase 1 + qb0 unit) so the exp-gated
    stretches of attention get PE filler; yt PSUM->SBUF copies alternate
    ACT/DVE; y stores go out as one wide DMA per token tile.
  * softmax close-out chains (reduce -> recip -> broadcast-mm -> normalize)
    are deferred into the following instruction stream so PE (in-order)
    never waits on them.
"""

import contextlib

import numpy as np

import concourse.bacc as bacc
import concourse.bass as bass
import concourse.mybir as mybir
import concourse.tile as tile
from concourse import bass_utils

B, T, D, H, DK = 2, 2048, 2048, 16, 128
NCORES = 8
HPC = H // NCORES  # heads per core
P = 128
NB = 512           # q-block / token-block / e-block width
KC = D // P        # 16 contraction chunks of the model dim
KP = KC // 2       # chunk pairs
QB = T // NB       # 4 q blocks per batch
NT = T // P        # 16 token tiles per batch
WC = 6 * P         # w columns per chunk: Q0 K0 Q1 K1 V0 V1
FP32 = mybir.dt.float32
F32R = mybir.dt.float32r
BF16 = mybir.dt.bfloat16
SCALE = 1.0 / np.sqrt(DK)

DEFAULT_OPTS = dict(
    ex_bufs=4, xt_bufs=2, yt_bufs=4, s_bufs=2, o_bufs=2, y_bufs=2,
    loop_n=1,
    # yt-copy engine cycle per *hosting location* of the proj units
    pat_attn={0: "DA", 1: "DA", 2: "DA", 3: "DA"},
    pat_p1="DA", pat_tail="DA",
    defer_pairs=3,
    chain_pat={0: "PD", 1: "PD", 2: "PD", 3: "DD"},  # per qb (even, odd pair)
    mask_eng="D",
    qkv_q="A", qkv_k="D", qkv_v="AD", pipe=True, chain_bf16=True,
)

_ENG_MAP = {"P": "gpsimd", "A": "scalar", "D": "vector"}


def _width(kt, qb):
    """Live q-suffix width of key tile kt within q-block qb (causal)."""
    j = kt - 4 * qb
    if j <= 0:
        return NB
    return NB - P * j


def build(debug=False, **opts):
    o = dict(DEFAULT_OPTS)
    o.update({k: v for k, v in opts.items() if k in DEFAULT_OPTS})
    nc = bacc.Bacc("TRN2", target_bir_lowering=False, debug=False,
                   num_devices=NCORES)
    # 3D dram layouts allow one DMA per chunk-pair / token tile
    xT = nc.dram_tensor("xT", [KC, P, B * T], BF16, kind="ExternalInput")
    w_d = nc.dram_tensor("w", [KC, P, WC], BF16, kind="ExternalInput")
    woT = nc.dram_tensor("woT", [HPC * DK, D], BF16, kind="ExternalInput")
    tri_d = nc.dram_tensor("tri", [P, P], BF16, kind="ExternalInput")
    onr_d = nc.dram_tensor("onr", [1, P], F32R, kind="ExternalInput")
    y = nc.dram_tensor("y", [B * T, D], BF16, kind="ExternalOutput")
    dbg = {}
    if debug:
        for nm in ("QT", "KT", "outT"):
            dbg[nm] = nc.dram_tensor(f"dbg_{nm}", [HPC, P, T], BF16,
                                     kind="ExternalOutput")
        dbg["V"] = nc.dram_tensor("dbg_V", [NT, P, 2 * P], BF16,
                                  kind="ExternalOutput")

    with tile.TileContext(nc) as tc:
        with (
            tc.tile_pool(name="const", bufs=1) as cpool,
            tc.tile_pool(name="xp", bufs=1) as xpool,
            tc.tile_pool(name="qkv", bufs=1) as qpool,
            tc.tile_pool(name="attn", bufs=1) as apool,
            tc.tile_pool(name="ps", bufs=1, space="PSUM") as pspool,
        ):
            # ---- constants / weights resident in SBUF ----
            w_sb = [cpool.tile([P, 2, WC], BF16, name=f"w_{kp}") for kp in range(KP)]
            wo_sb = [cpool.tile([P, D], BF16, name=f"wo_{h}") for h in range(HPC)]
            tri = cpool.tile([P, P], BF16, name="tri")
            onr = cpool.tile([1, P], F32R, name="onr")

            def wqk(k, m):  # m in 0..3 = Q0 K0 Q1 K1 of chunk k
                return w_sb[k // 2][:, k % 2, m * P:(m + 1) * P]

            def wvv(k):     # V columns (both heads) of chunk k
                return w_sb[k // 2][:, k % 2, 4 * P:6 * P]

            # persistent per-batch state (WAR deps recycle across batches)
            QTp = [qpool.tile([P, T], BF16, name=f"QT{h}") for h in range(HPC)]
            KTp = [qpool.tile([P, T], BF16, name=f"KT{h}") for h in range(HPC)]
            Vp = [qpool.tile([P, 2 * P], BF16, name=f"V{kt}") for kt in range(NT)]
            outTp = [qpool.tile([P, T], BF16, name=f"outT{h}") for h in range(HPC)]

            def ps_tile(tag, shape, name, bufs):
                return pspool.tile(shape, FP32, name=name, tag=tag, bufs=bufs)

            def copy_on(code, dst, src):
                eng = _ENG_MAP[code]
                if eng == "gpsimd":
                    nc.gpsimd.tensor_copy(dst, src)
                elif eng == "scalar":
                    nc.scalar.copy(dst, src)
                else:
                    nc.vector.tensor_copy(dst, src)

            loop_ctx = (tc.For_i(0, o["loop_n"], 1, hint_engines=(
                            mybir.EngineType.PE, mybir.EngineType.Activation,
                            mybir.EngineType.DVE, mybir.EngineType.SP,
                            mybir.EngineType.Pool))
                        if o["loop_n"] > 1 else contextlib.nullcontext())

            if o["loop_n"] > 1:
                # weights/constants loaded once, outside the HW loop
                for kp in range(KP):
                    nc.sync.dma_start(w_sb[kp][:], w_d[2 * kp:2 * kp + 2])
                nc.sync.dma_start(tri[:], tri_d[:, :])
                nc.sync.dma_start(onr[:], onr_d[:, :])
                for h in range(HPC):
                    nc.sync.dma_start(wo_sb[h][:], woT[h * P:(h + 1) * P, :])

            with loop_ctx:
                # ============ phase 1 generator (one token block) ============
                def load_nb(b, nb):
                    """Allocate + DMA the x tiles for token block (b, nb)."""
                    col0 = b * T + nb * NB
                    xt = [xpool.tile([P, 2, NB], BF16, name=f"x{kp}_{b}_{nb}",
                                     tag=f"x{kp}", bufs=o["xt_bufs"])
                          for kp in range(KP)]
                    first = b == 0 and nb == 0 and o["loop_n"] == 1
                    for kp in range(KP):
                        if first and kp == 0:
                            # column-split the first w pair-DMA (row interleave
                            # preserved) and slot x0 between the halves so the
                            # first Q/K matmuls unblock after w0a + x0
                            nc.sync.dma_start(w_sb[0][:, :, 0:2 * P],
                                              w_d[0:2, :, 0:2 * P])
                            nc.sync.dma_start(
                                xt[0][:], xT[0:2, :, col0:col0 + NB])
                            nc.sync.dma_start(w_sb[0][:, :, 2 * P:WC],
                                              w_d[0:2, :, 2 * P:WC])
                            continue
                        if first:
                            nc.sync.dma_start(w_sb[kp][:], w_d[2 * kp:2 * kp + 2])
                        nc.sync.dma_start(xt[kp][:],
                                          xT[2 * kp:2 * kp + 2, :, col0:col0 + NB])
                        if first and kp == 1:
                            nc.sync.dma_start(tri[:], tri_d[:, :])
                            nc.sync.dma_start(onr[:], onr_d[:, :])
                    return xt

                def phase1_nb(b, nb, xt, mid=None):
                    """QKV projection for token block (b, nb), k-outer.
                    Yields after each chunk-pair (8) + drains; `mid` thunk
                    (next-block prefetch) fires after chunk-pair 5."""
                    if b == 0 and nb == 1 and o["loop_n"] == 1:
                        for h in range(HPC):
                            nc.sync.dma_start(wo_sb[h][:], woT[h * P:(h + 1) * P, :])

                    S0 = ps_tile("S", [P, 2, NB], f"p1s0_{b}_{nb}", o["s_bufs"])
                    S1 = ps_tile("S", [P, 2, NB], f"p1s1_{b}_{nb}", o["s_bufs"])
                    # V token-tile groups need a PSUM bank each (one
                    # accumulation group per bank): two sub-sweeps of 2.
                    V01 = [ps_tile("O", [P, NB], f"p1v{t}_{b}_{nb}", o["o_bufs"])
                           for t in range(2)]
                    for kp in range(KP):
                        for half in range(2):
                            k = 2 * kp + half
                            st, sp = k == 0, k == KC - 1
                            xk = xt[kp][:, half, :]
                            nc.tensor.matmul(S0[:, 0, :], wqk(k, 0), xk,
                                             start=st, stop=sp)
                            nc.tensor.matmul(S0[:, 1, :], wqk(k, 1), xk,
                                             start=st, stop=sp)
                            nc.tensor.matmul(S1[:, 0, :], wqk(k, 2), xk,
                                             start=st, stop=sp)
                            nc.tensor.matmul(S1[:, 1, :], wqk(k, 3), xk,
                                             start=st, stop=sp)
                            for t in range(2):
                                nc.tensor.matmul(V01[t][:, 0:2 * P],
                                                 xt[kp][:, half, t * P:(t + 1) * P],
                                                 wvv(k), start=st, stop=sp)
                        if kp == 5 and mid is not None:
                            mid()
                        yield
                    # drain V first (V2/V3 sweeps wait on these PSUM
                    # banks), then QK (next block's S tiles are far off)
                    csl = slice(nb * NB, (nb + 1) * NB)
                    for t in range(2):
                        copy_on(o["qkv_v"][t % len(o["qkv_v"])],
                                Vp[nb * 4 + t][:], V01[t][:, 0:2 * P])
                    copy_on(o["qkv_q"], QTp[0][:, csl], S0[:, 0, :])
                    copy_on(o["qkv_k"], KTp[0][:, csl], S0[:, 1, :])
                    copy_on(o["qkv_q"], QTp[1][:, csl], S1[:, 0, :])
                    copy_on(o["qkv_k"], KTp[1][:, csl], S1[:, 1, :])
                    yield
                    # V2 then V3 sequentially: each holds only ONE O slot,
                    # so attention(qb0) can interleave using the other slot
                    for t in range(2, 4):
                        Vt = ps_tile("O", [P, NB], f"p1v{t}_{b}_{nb}",
                                     o["o_bufs"])
                        for kp in range(KP):
                            for half in range(2):
                                k = 2 * kp + half
                                nc.tensor.matmul(Vt[:, 0:2 * P],
                                                 xt[kp][:, half,
                                                        t * P:(t + 1) * P],
                                                 wvv(k), start=(k == 0),
                                                 stop=(k == KC - 1))
                            if kp % 2 == 1:
                                yield
                        copy_on(o["qkv_v"][t % len(o["qkv_v"])],
                                Vp[nb * 4 + t][:], Vt[:, 0:2 * P])
                        yield

                # ============ attention generator (one head) ============
                def attention_gen(b, h, qb):
                    """Yields once per kt-pair.  Returns the deferred
                    close-out thunk (bcmm + normalize)."""
                    nkt = 4 * qb + 4
                    qsl0 = qb * NB
                    ps_o = ps_tile("O", [P, NB], f"pso_{b}_{h}_{qb}", o["o_bufs"])
                    # two running-sum chains: even pairs / odd pairs
                    cdt = BF16 if o["chain_bf16"] else FP32
                    exs = [apool.tile([P, NB], cdt, name=f"exs{i}_{b}_{h}_{qb}",
                                      tag=f"exsum{i}", bufs=2) for i in range(2)]
                    npair = nkt // 2

                    def emit_scores_exp(p):
                        a, c = 2 * p, 2 * p + 1
                        oa, oc = NB - _width(a, qb), NB - _width(c, qb)
                        ps_s = ps_tile("S", [P, 2, NB], f"pss_{b}_{h}_{qb}_{p}",
                                       o["s_bufs"])
                        nc.tensor.matmul(ps_s[:, 0, oa:NB],
                                         KTp[h][:, a * P:(a + 1) * P],
                                         QTp[h][:, qsl0 + oa:qsl0 + NB],
                                         start=True, stop=True)
                        nc.tensor.matmul(ps_s[:, 1, oc:NB],
                                         KTp[h][:, c * P:(c + 1) * P],
                                         QTp[h][:, qsl0 + oc:qsl0 + NB],
                                         start=True, stop=True)
                        ex = apool.tile([P, 2, NB], BF16,
                                        name=f"ex_{b}_{h}_{qb}_{p}",
                                        tag="ex", bufs=o["ex_bufs"])
                        if oa == oc:
                            nc.scalar.activation(ex[:, :, oa:NB],
                                                 ps_s[:, :, oa:NB],
                                                 mybir.ActivationFunctionType.Exp,
                                                 scale=SCALE)
                        else:
                            nc.scalar.activation(ex[:, 0, oa:NB],
                                                 ps_s[:, 0, oa:NB],
                                                 mybir.ActivationFunctionType.Exp,
                                                 scale=SCALE)
                            nc.scalar.activation(ex[:, 1, oc:NB],
                                                 ps_s[:, 1, oc:NB],
                                                 mybir.ActivationFunctionType.Exp,
                                                 scale=SCALE)
                        return (p, ex, oa, oc)

                    def emit_post(st):
                        p, ex, oa, oc = st
                        a, c = 2 * p, 2 * p + 1
                        # zero dead prefixes of narrowed (diagonal) tiles
                        if oa > 0:
                            nc.gpsimd.memset(ex[:, 0, 0:oa], 0.0)
                        if oc > 0:
                            nc.gpsimd.memset(ex[:, 1, 0:oc], 0.0)
                        # triangle masks on diagonal tiles
                        for half, kt, off in ((0, a, oa), (1, c, oc)):
                            if kt >= 4 * qb:
                                sl = ex[:, half, off:off + P]
                                if o["mask_eng"] == "P":
                                    nc.gpsimd.tensor_mul(sl, sl, tri[:])
                                else:
                                    nc.vector.tensor_mul(sl, sl, tri[:])
                        # denominator partial: exs[p%2] += ex.lo + ex.hi
                        tpr = apool.tile([P, NB], BF16,
                                         name=f"tp_{b}_{h}_{qb}_{p}",
                                         tag="tpr", bufs=2)
                        nc.vector.tensor_add(tpr[:], ex[:, 0, :], ex[:, 1, :])
                        cp = o["chain_pat"][qb] if isinstance(o["chain_pat"], dict) else o["chain_pat"]
                        eng = getattr(nc, _ENG_MAP[cp[p % 2]])
                        if p < 2:
                            eng.tensor_copy(exs[p % 2][:], tpr[:])
                        else:
                            eng.tensor_add(exs[p % 2][:], exs[p % 2][:], tpr[:])
                        # AV accumulation
                        nc.tensor.matmul(ps_o[:, oa:NB],
                                         Vp[a][:, h * P:(h + 1) * P],
                                         ex[:, 0, oa:NB],
                                         start=(p == 0), stop=False,
                                         skip_group_check=True)
                        nc.tensor.matmul(ps_o[:, oc:NB],
                                         Vp[c][:, h * P:(h + 1) * P],
                                         ex[:, 1, oc:NB],
                                         start=False, stop=(p == npair - 1),
                                         skip_group_check=True)

                    # software pipeline: scores/exp of p+1 before AV of p
                    if o["pipe"]:
                        st = emit_scores_exp(0)
                        for p in range(npair):
                            nxt = (emit_scores_exp(p + 1)
                                   if p + 1 < npair else None)
                            emit_post(st)
                            st = nxt
                            yield
                    else:
                        for p in range(npair):
                            emit_post(emit_scores_exp(p))
                            yield
                    # denominator: merge chains, cross-partition reduce, recip
                    rec = apool.tile([1, NB], F32R, name=f"rec_{b}_{h}_{qb}",
                                     tag="rec", bufs=2)
                    dn = apool.tile([1, NB], FP32, name=f"dn_{b}_{h}_{qb}",
                                    tag="dn", bufs=2)
                    fin = b == B - 1 and qb == QB - 1 and h == 1
                    if o["chain_bf16"]:
                        mrg = apool.tile([P, NB], BF16, name=f"mg_{b}_{h}_{qb}",
                                         tag="mrg", bufs=2)
                        if fin:
                            # final close: column-halves so the tail proj can
                            # start on half 0 while half 1 still reduces
                            for cs in (slice(0, NB // 2), slice(NB // 2, NB)):
                                nc.vector.tensor_add(mrg[:, cs], exs[0][:, cs],
                                                     exs[1][:, cs])
                                nc.gpsimd.tensor_reduce(
                                    dn[:1, cs], mrg[:, cs],
                                    axis=mybir.AxisListType.C,
                                    op=mybir.AluOpType.add)
                        else:
                            nc.vector.tensor_add(mrg[:], exs[0][:], exs[1][:])
                            nc.gpsimd.tensor_reduce(dn[:1, :], mrg[:],
                                                    axis=mybir.AxisListType.C,
                                                    op=mybir.AluOpType.add)
                    else:
                        dn1 = apool.tile([1, NB], FP32, name=f"dn1_{b}_{h}_{qb}",
                                         tag="dn1", bufs=2)
                        nc.gpsimd.tensor_reduce(dn[:1, :], exs[0][:],
                                                axis=mybir.AxisListType.C,
                                                op=mybir.AluOpType.add)
                        nc.gpsimd.tensor_reduce(dn1[:1, :], exs[1][:],
                                                axis=mybir.AxisListType.C,
                                                op=mybir.AluOpType.add)
                        nc.vector.tensor_add(dn[:1, :], dn[:1, :], dn1[:1, :])
                    with nc.allow_low_precision(reason="f32r recip: tf32 ok"):
                        if fin:
                            nc.vector.reciprocal(rec[:1, 0:NB // 2],
                                                 dn[:1, 0:NB // 2])
                            nc.vector.reciprocal(rec[:1, NB // 2:NB],
                                                 dn[:1, NB // 2:NB])
                        else:
                            nc.vector.reciprocal(rec[:1, :], dn[:1, :])

                    def close():
                        ps_bc = ps_tile("Y", [P, NB], f"psbc_{b}_{h}_{qb}",
                                        o["y_bufs"])
                        bc = apool.tile([P, NB], FP32, name=f"bc_{b}_{h}_{qb}",
                                        tag="bc", bufs=2)
                        slices = ((slice(0, NB // 2), slice(NB // 2, NB))
                                  if fin else (slice(0, NB),))
                        for cs in slices:
                            nc.tensor.matmul(ps_bc[:, cs], onr[:1, :],
                                             rec[:1, cs], start=True, stop=True)
                            nc.vector.tensor_copy(bc[:, cs], ps_bc[:, cs])
                            nc.vector.tensor_mul(
                                outTp[h][:, qsl0 + cs.start:qsl0 + cs.stop],
                                ps_o[:, cs], bc[:, cs])
                    return close

                # ============ proj units ============
                yts = {}

                def proj_unit(b, tt, eb, eng_code, narrow_dma=False,
                              tag="Y"):
                    ps_y = ps_tile(tag, [P, NB], f"psy_{b}_{tt}_{eb}",
                                   o["y_bufs"])
                    nc.tensor.matmul(ps_y[:], outTp[0][:, tt * P:(tt + 1) * P],
                                     wo_sb[0][:, eb * NB:(eb + 1) * NB],
                                     start=True, stop=False)
                    nc.tensor.matmul(ps_y[:], outTp[1][:, tt * P:(tt + 1) * P],
                                     wo_sb[1][:, eb * NB:(eb + 1) * NB],
                                     start=False, stop=True)
                    if eb == 0:
                        yts[(b, tt)] = apool.tile([P, QB, NB], BF16,
                                                  name=f"yt_{b}_{tt}",
                                                  tag="yt", bufs=o["yt_bufs"])
                    yt = yts[(b, tt)]
                    copy_on(eng_code, yt[:, eb, :], ps_y[:])
                    if narrow_dma:
                        # tail: stream each e-block out as soon as copied
                        nc.sync.dma_start(
                            y[b * T + tt * P:b * T + (tt + 1) * P,
                              eb * NB:(eb + 1) * NB], yt[:, eb, :])
                    elif eb == QB - 1:
                        nc.sync.dma_start(
                            y[b * T + tt * P:b * T + (tt + 1) * P, :], yt[:])

                def proj_thunks(b, pqb, pat, narrow_dma=False, tags="Y"):
                    th = []
                    i = 0
                    for tt in range(4 * pqb, 4 * pqb + 4):
                        for eb in range(QB):
                            code = pat[i % len(pat)]
                            tag = tags[i % len(tags)]
                            th.append(lambda b=b, tt=tt, eb=eb, code=code,
                                      tag=tag:
                                      proj_unit(b, tt, eb, code, narrow_dma,
                                                tag))
                            i += 1
                    return th

                # ============ weaving driver ============
                def weave(gen, fillers, carry, defer=None):
                    """Run gen; after each yield emit carry thunks (once,
                    after o['defer_pairs'] yields) and a fair share of
                    fillers (popped from the shared list)."""
                    n = 0
                    held = 0
                    try:
                        while True:
                            next(gen)
                            n += 1
                            if n >= o["defer_pairs"] and carry:
                                for fn in carry:
                                    fn()
                                carry = []
                            if defer:
                                held += defer.pop(0)
                            if not carry:
                                while held > 0 and fillers:
                                    fillers.pop(0)()
                                    held -= 1
                    except StopIteration as si:
                        for fn in carry:
                            fn()
                        return si.value

                def share(nfill, nsteps):
                    base, rem = divmod(nfill, nsteps)
                    return [base + (1 if i < rem else 0) for i in range(nsteps)]

                def attention_unit(b, qb, fillers, carry, reserve=3,
                                   final=False):
                    npair = 2 * qb + 2
                    # hold a few fillers back to cover the close-out chain
                    # latency after the last AV pair
                    nres = min(reserve, len(fillers))
                    sh = share(len(fillers) - nres, 2 * npair)
                    close0 = weave(attention_gen(b, 0, qb), fillers, carry,
                                   defer=sh[:npair])
                    close1 = weave(attention_gen(b, 1, qb), fillers, [close0],
                                   defer=sh[npair:])
                    if final:
                        for fn in fillers:
                            fn()
                        del fillers[:]
                        close1()
                        return []
                    for fn in fillers:  # reserved + leftovers
                        fn()
                    del fillers[:]
                    return [close1]

                def prefix(gen, n):
                    for _ in range(n):
                        next(gen)
                        yield

                # ============ main schedule ============
                carry = []
                xts = {}
                for b in range(B):
                    # ---- phase 1 (+ second half of prev batch qb3 proj) ----
                    if b > 0:
                        ph1_fill = proj_thunks(b - 1, 3, o["pat_p1"])[8:]
                        qb0_fill = proj_thunks(b - 1, 3, o["pat_attn"][3])[:8]
                    else:
                        ph1_fill, qb0_fill = [], []
                    if b == 0:
                        xts["cur"] = load_nb(0, 0)
                    for nb in range(QB - 1):
                        nxt = [b, nb + 1]
                        mid = (lambda nxt=nxt:
                               xts.__setitem__("next", load_nb(*nxt)))
                        weave(phase1_nb(b, nb, xts["cur"], mid), ph1_fill,
                              carry,
                              defer=share(2, KP + 1) if ph1_fill else None)
                        xts["cur"] = xts["next"]
                        carry = []
                    # nb3: QK sweep + V01; the V2/V3 tail becomes PE filler
                    # for the attention(qb0) unit
                    mid = ((lambda: xts.__setitem__("next", load_nb(b + 1, 0)))
                           if b + 1 < B else None)
                    g_ph = phase1_nb(b, 3, xts["cur"], mid)
                    weave(prefix(g_ph, 9), ph1_fill, carry,
                          defer=share(2, 10) if ph1_fill else None)
                    if b + 1 < B:
                        xts["cur"] = xts["next"]
                    carry = []
                    for fn in ph1_fill:
                        fn()
                    tails = [(lambda: next(g_ph, None)) for _ in range(10)]
                    # ---- qb0 unit merged with phase-1 V tail ----
                    # one V2 step first fixes the O-slot rotation so the V3
                    # sweep can safely cover the h0 close-out chain
                    tails.pop(0)()
                    close0 = weave(attention_gen(b, 0, 0), tails, carry,
                                   defer=[2, 2])
                    for _ in range(3):  # V3 progress covers the dn/recip chain
                        if tails:
                            tails.pop(0)()
                    close0()
                    fill2 = tails + qb0_fill
                    close1 = weave(attention_gen(b, 1, 0), fill2,
                                   [], defer=[2, 2])
                    for fn in fill2:
                        fn()
                    carry = [close1]
                    carry = attention_unit(
                        b, 1, proj_thunks(b, 0, o["pat_attn"][0]), carry)
                    carry = attention_unit(
                        b, 2, proj_thunks(b, 1, o["pat_attn"][1]), carry)
                    carry = attention_unit(
                        b, 3, proj_thunks(b, 2, o["pat_attn"][2]
                                          if b < B - 1
                                          else "DADADADADA" + "A" * 6), carry,
                        reserve=8, final=(b == B - 1))
                # ---- tail: close-out then final batch qb3 proj ----
                for c in carry:
                    c()
                carry = []
                for fn in proj_thunks(B - 1, 3, o["pat_tail"],
                                      narrow_dma=False, tags="YO"):
                    fn()
                if debug:
                    for h in range(HPC):
                        nc.sync.dma_start(dbg["QT"][h], QTp[h][:])
                        nc.sync.dma_start(dbg["KT"][h], KTp[h][:])
                        nc.sync.dma_start(dbg["outT"][h], outTp[h][:])
                    for kt in range(NT):
                        nc.sync.dma_start(dbg["V"][kt], Vp[kt][:])

    nc.compile()
    return nc


def prep_inputs(x, w_qkv, w_o):
    """Host-side shard prep. Returns per-core input maps (bf16)."""
    bf = mybir.dt.np(BF16)
    x = np.asarray(x, dtype=np.float32).reshape(B * T, D)
    xT = np.ascontiguousarray(x.T).reshape(KC, P, B * T).astype(bf)
    w_qkv = np.asarray(w_qkv, dtype=np.float32)
    w_o = np.asarray(w_o, dtype=np.float32)

    tri = np.zeros((P, P), dtype=np.float32)
    kp = np.arange(P)[:, None]
    qu = np.arange(P)[None, :]
    tri[kp <= qu] = 1.0
    tri = tri.astype(bf)
    onr = np.ones((1, P), dtype=np.float32)

    in_maps = []
    for c in range(NCORES):
        h0, h1 = HPC * c, HPC * c + 1
        cols = []
        for h in (h0, h1):
            cols += [w_qkv[h * DK:(h + 1) * DK],            # Q rows
                     w_qkv[D + h * DK:D + (h + 1) * DK]]    # K rows
        # reorder to Q0 K0 Q1 K1 then V0 V1
        cols = [cols[0], cols[1], cols[2], cols[3],
                w_qkv[2 * D + h0 * DK:2 * D + (h0 + 1) * DK],
                w_qkv[2 * D + h1 * DK:2 * D + (h1 + 1) * DK]]
        w = np.ascontiguousarray(
            np.concatenate(cols, 0).T).reshape(KC, P, WC).astype(bf)
        wo = np.ascontiguousarray(
            w_o[:, HPC * DK * c:HPC * DK * (c + 1)].T).astype(bf)
        in_maps.append({
            "xT": xT, "w": w, "woT": wo, "tri": tri, "onr": onr,
        })
    return in_maps


_nc_cache = {}


def get_nc(debug=False, **opts):
    key = (debug, tuple(sorted((k, str(v)) for k, v in opts.items())))
    if key not in _nc_cache:
        _nc_cache[key] = build(debug=debug, **opts)
    return _nc_cache[key]


def run(x, w_qkv, w_o, debug=False, **opts):
    nc = get_nc(debug=debug, **opts)
    in_maps = prep_inputs(x, w_qkv, w_o)
    res = bass_utils.run_bass_kernel_spmd(nc, in_maps, core_ids=list(range(NCORES)))
    return res


def kernel(x, w_qkv, w_o):
    res = run(x, w_qkv, w_o)
    y = res.results[0]["y"].astype(np.float64)
    for c in range(1, NCORES):
        y += res.results[c]["y"]
    return y.astype(np.float32).reshape(B, T, D)



# revision 21
# speedup vs baseline: 1.1545x; 1.1545x over previous
"""Trainium2 Bass kernel for causal multi-head attention (dense transformer block).

Math (reference semantics):
    qkv = x @ w_qkv.T ; split into Q,K,V heads [B,H,T,dk]
    (rotary in the reference rotates Q and K of head h by a constant,
     time-independent orthogonal rotation R_h; since scores = (R_h q)·(R_h k)
     = q·k, the rotation cancels exactly and is skipped here)
    scores = causal_mask(Q @ K.T / sqrt(dk)); attn = softmax(scores)
    out = attn @ V ; y = out @ w_o.T

Sharding: head-parallel over 8 cores (2 heads/core, both batches).  Each core
computes a partial y (its heads' contribution through w_o columns); the host
sums the 8 partials (the "all-reduce").

v4 design (vs the bf16 v3 baseline):
  * QKV projection and out-projection run as 3-term compensated fp8
    (e4m3) DoubleRow matmuls: operands are split hi+lo (lo is the
    quantization residual, representable via fp8 subnormals without any
    rescale), and x@w = xh@wh + xl@wh + xh@wl accumulates in fp32 PSUM.
    DoubleRow contracts 2 k-tiles (256) per instruction at 0.5 cyc/row,
    so the three terms cost 0.75x the bf16 schedule on PE while being
    MORE accurate than bf16 (the residual kills the quantization error).
  * hi and lo planes are packed in one dram tensor so each chunk-pair /
    weight tile still costs ONE hwdge descriptor-generation slot (the
    dge fixed cost, not bytes, is the phase-1 DMA limiter).
  * Scale plumbing (no extra device ops): Q/K rows of w_qkv and all of
    w_o are scaled x64 host-side so their fp8 mantissas are in range
    (the exp scale absorbs the resulting x4096 on scores); V rows are
    scaled x32 so outT = ps_o * (1/denom) lands at x32 with good fp8
    hi/lo precision; y carries x32x64 and the host divides by 2048.
  * Attention (scores, exp, AV) stays bf16: plain-fp8 Q/K/ex/V all
    overshoot the error budget and compensated fp8 is slower than bf16
    at dk=128 contraction.
  * Softmax close-out: GpSimd partition_all_reduce folds the
    cross-partition denominator sum and its broadcast into one op, so
    DVE reciprocal directly yields the [P,NB] normalize multiplier (no
    PE ones-matmul); the normalized outT is written as fp8 hi + fp8
    residual lo feeding the DoubleRow out-projection.
  * DMA rule (hard-won): dma_start pairs elements by flat AP iteration
    order, NOT by dimension names.  Every dram tensor here is laid out
    so a DMA slice iterates exactly like its destination SBUF tile
    ([kp, p, hi/lo, parity, cols]); getting this wrong scrambles rows
    silently (the all-hi term still "works" because both operands get
    the same scramble -- only cross terms and the out-proj break).
  * Tail: the last batch's qb3 close-outs write bf16 outT directly
    (short chain, no fp8 split), quartered by token columns, and the
    final 16 proj units run bf16 2-matmul style interleaved with the
    quarters so PE restarts ~3us earlier.
  * Schedule (weaving of proj units into attention kt-pairs, deferred
    close-out chains, chunk-pair DMAs, causal narrowing with GpSimd
    memset/triangle masks) is inherited from v3.
"""

import contextlib

import numpy as np

import concourse.bacc as bacc
import concourse.bass as bass
import concourse.bass_isa as bass_isa
import concourse.mybir as mybir
import concourse.tile as tile
from concourse import bass_utils

B, T, D, H, DK = 2, 2048, 2048, 16, 128
NCORES = 8
HPC = H // NCORES  # heads per core
P = 128
NB = 512           # q-block / token-block / e-block width
KC = D // P        # 16 contraction chunks of the model dim
KP = KC // 2       # chunk pairs
QB = T // NB       # 4 q blocks per batch
NT = T // P        # 16 token tiles per batch
WC = 6 * P         # w columns per chunk: Q0 K0 Q1 K1 V0 V1
FP32 = mybir.dt.float32
F32R = mybir.dt.float32r
BF16 = mybir.dt.bfloat16
F8 = mybir.dt.float8e4
DR = mybir.MatmulPerfMode.DoubleRow
WSCALE = 64.0      # host scale on w_qkv Q/K rows and w_o (fp8 range)
VSCALE = 32.0      # host scale on w_qkv V rows => outT carries x32
YDIV = VSCALE * WSCALE  # host divides gathered y by this
SCALE = 1.0 / np.sqrt(DK)
ESCALE = SCALE / (WSCALE * WSCALE)  # exp() input scale: Q,K carry x64 each

DEFAULT_OPTS = dict(
    ex_bufs=6, xt_bufs=2, yt_bufs=4, s_bufs=2, o_bufs=2, y_bufs=2,
    loop_n=1,
    # yt-copy engine cycle per *hosting location* of the proj units
    pat_attn={0: "DAA", 1: "DA", 2: "DA", 3: "DAA"},
    pat_p1="DA", pat_tail="AD",
    defer_pairs=4,
    chain_pat={0: "DD", 1: "PD", 2: "PD", 3: "PD"},  # per qb (even, odd pair)
    mask_eng="D",
    qkv_q="A", qkv_k="D", qkv_v="DD", pipe=True, chain_bf16=True,
    oh_eng="D",     # outH fp8 copy engine
    tpr_eng="D",    # per-pair ex.lo+ex.hi add engine
    alrd=True,      # partition_all_reduce denominator (no broadcast stage)
    reserve3=6,     # fillers reserved past the final attention weave
    reserve=3,      # fillers reserved in non-final attention units
    tail_tags="YO",
)

_ENG_MAP = {"P": "gpsimd", "A": "scalar", "D": "vector"}


def _width(kt, qb):
    """Live q-suffix width of key tile kt within q-block qb (causal)."""
    j = kt - 4 * qb
    if j <= 0:
        return NB
    return NB - P * j


def build(debug=False, **opts):
    o = dict(DEFAULT_OPTS)
    o.update({k: v for k, v in opts.items() if k in DEFAULT_OPTS})
    nc = bacc.Bacc("TRN2", target_bir_lowering=False, debug=False,
                   num_devices=NCORES)
    # 3D dram layouts allow one DMA per chunk-pair / token tile; dim 0
    # packs the fp8 hi/lo planes so one DMA moves both.
    # dim order matches the SBUF tile iteration order exactly (dma_start
    # pairs elements by flat AP order): [kp, p, hi/lo, chunk-parity, cols]
    x8 = nc.dram_tensor("x8", [KP, P, 2, 2, B * T], F8, kind="ExternalInput")
    w8 = nc.dram_tensor("w8", [KP, P, 2, 2, WC], F8, kind="ExternalInput")
    wo8 = nc.dram_tensor("wo8", [P, 2, HPC, D], F8, kind="ExternalInput")
    wobf_d = nc.dram_tensor("wobf", [HPC, P, D], BF16, kind="ExternalInput")
    tri_d = nc.dram_tensor("tri", [P, P], BF16, kind="ExternalInput")
    onr_d = nc.dram_tensor("onr", [1, P], F32R, kind="ExternalInput")
    y = nc.dram_tensor("y", [B * T, D], BF16, kind="ExternalOutput")
    dbg = {}
    if debug:
        for nm in ("QT", "KT"):
            dbg[nm] = nc.dram_tensor(f"dbg_{nm}", [HPC, P, T], BF16,
                                     kind="ExternalOutput")
        dbg["outH"] = nc.dram_tensor("dbg_outH", [P, HPC, T], F8,
                                     kind="ExternalOutput")
        dbg["outL"] = nc.dram_tensor("dbg_outL", [P, HPC, T], F8,
                                     kind="ExternalOutput")
        dbg["V"] = nc.dram_tensor("dbg_V", [NT, P, 2 * P], BF16,
                                  kind="ExternalOutput")

    with tile.TileContext(nc) as tc:
        with (
            tc.tile_pool(name="const", bufs=1) as cpool,
            tc.tile_pool(name="xp", bufs=1) as xpool,
            tc.tile_pool(name="qkv", bufs=1) as qpool,
            tc.tile_pool(name="attn", bufs=1) as apool,
            tc.tile_pool(name="ps", bufs=1, space="PSUM") as pspool,
        ):
            # ---- constants / weights resident in SBUF ----
            # dim1 = hi/lo plane, dim2 = chunk parity (DoubleRow k-tiles)
            w_sb = [cpool.tile([P, 2, 2, WC], F8, name=f"w_{kp}")
                    for kp in range(KP)]
            wo_sb = cpool.tile([P, 2, HPC, D], F8, name="wo")
            wobf_sb = [cpool.tile([P, D], BF16, name=f"wobf{h}")
                       for h in range(HPC)]
            tri = cpool.tile([P, P], BF16, name="tri")
            onr = cpool.tile([1, P], F32R, name="onr")

            # persistent per-batch state (WAR deps recycle across batches)
            QTp = [qpool.tile([P, T], BF16, name=f"QT{h}") for h in range(HPC)]
            KTp = [qpool.tile([P, T], BF16, name=f"KT{h}") for h in range(HPC)]
            Vp = [qpool.tile([P, 2 * P], BF16, name=f"V{kt}") for kt in range(NT)]
            outH = qpool.tile([P, HPC, T], F8, name="outH")
            outL = qpool.tile([P, HPC, T], F8, name="outL")
            outTb = qpool.tile([P, HPC, NB], BF16, name="outTb")  # tail qb3

            def ps_tile(tag, shape, name, bufs):
                return pspool.tile(shape, FP32, name=name, tag=tag, bufs=bufs)

            def copy_on(code, dst, src):
                eng = _ENG_MAP[code]
                if eng == "gpsimd":
                    nc.gpsimd.tensor_copy(dst, src)
                elif eng == "scalar":
                    nc.scalar.copy(dst, src)
                else:
                    nc.vector.tensor_copy(dst, src)

            loop_ctx = (tc.For_i(0, o["loop_n"], 1, hint_engines=(
                            mybir.EngineType.PE, mybir.EngineType.Activation,
                            mybir.EngineType.DVE, mybir.EngineType.SP,
                            mybir.EngineType.Pool))
                        if o["loop_n"] > 1 else contextlib.nullcontext())

            if o["loop_n"] > 1:
                # weights/constants loaded once, outside the HW loop
                for kp in range(KP):
                    nc.sync.dma_start(w_sb[kp][:], w8[kp])
                nc.sync.dma_start(tri[:], tri_d[:, :])
                nc.sync.dma_start(onr[:], onr_d[:, :])
                nc.sync.dma_start(wo_sb[:], wo8[:, :, :, :])
                for h in range(HPC):
                    nc.sync.dma_start(wobf_sb[h][:], wobf_d[h])

            with loop_ctx:
                # ============ phase 1 generator (one token block) ============
                def load_nb(b, nb):
                    """Allocate + DMA the x tiles for token block (b, nb)."""
                    col0 = b * T + nb * NB
                    xt = [xpool.tile([P, 2, 2, NB], F8, name=f"x{kp}_{b}_{nb}",
                                     tag=f"x{kp}", bufs=o["xt_bufs"])
                          for kp in range(KP)]
                    first = b == 0 and nb == 0 and o["loop_n"] == 1
                    for kp in range(KP):
                        if first and kp == 0:
                            # split the first loads so the first Q/K matmuls
                            # unblock after the hi Q0K0 columns + hi x0
                            nc.sync.dma_start(w_sb[0][:, 0, :, 0:2 * P],
                                              w8[0, :, 0, :, 0:2 * P])
                            nc.sync.dma_start(
                                xt[0][:, 0],
                                x8[0, :, 0, :, col0:col0 + NB])
                            nc.sync.dma_start(w_sb[0][:, 0, :, 2 * P:WC],
                                              w8[0, :, 0, :, 2 * P:WC])
                            nc.sync.dma_start(
                                xt[0][:, 1],
                                x8[0, :, 1, :, col0:col0 + NB])
                            nc.sync.dma_start(w_sb[0][:, 1], w8[0, :, 1])
                            continue
                        if first:
                            nc.sync.dma_start(w_sb[kp][:], w8[kp])
                        nc.sync.dma_start(
                            xt[kp][:], x8[kp, :, :, :, col0:col0 + NB])
                        if first and kp == 1:
                            nc.sync.dma_start(tri[:], tri_d[:, :])
                            nc.sync.dma_start(onr[:], onr_d[:, :])
                    return xt

                # (w_hl, x_hl) term order: hi@hi, hi-w@lo-x, lo-w@hi-x
                TERMS = ((0, 0), (0, 1), (1, 0))

                def phase1_nb(b, nb, xt, mid=None):
                    """QKV projection for token block (b, nb), k-outer,
                    3-term compensated fp8 DoubleRow.  Yields after each
                    chunk-pair (8) + drains; `mid` thunk (next-block
                    prefetch) fires after chunk-pair 5."""
                    if b == 0 and nb == 1 and o["loop_n"] == 1:
                        nc.sync.dma_start(wo_sb[:], wo8[:, :, :, :])
                    if b == 1 and nb == 1 and o["loop_n"] == 1:
                        for h in range(HPC):
                            nc.sync.dma_start(wobf_sb[h][:], wobf_d[h])

                    S0 = ps_tile("S", [P, 2, NB], f"p1s0_{b}_{nb}", o["s_bufs"])
                    S1 = ps_tile("S", [P, 2, NB], f"p1s1_{b}_{nb}", o["s_bufs"])
                    # V token-tile groups need a PSUM bank each (one
                    # accumulation group per bank): two sub-sweeps of 2.
                    V01 = [ps_tile("O", [P, NB], f"p1v{t}_{b}_{nb}", o["o_bufs"])
                           for t in range(2)]
                    SQK = (S0[:, 0, :], S0[:, 1, :], S1[:, 0, :], S1[:, 1, :])
                    first_nb = b == 0 and nb == 0 and o["loop_n"] == 1
                    for kp in range(KP):
                        for ti, (wi, xi) in enumerate(TERMS):
                            st = kp == 0 and ti == 0
                            sp = kp == KP - 1 and ti == 2
                            for m in range(4):
                                nc.tensor.matmul(
                                    SQK[m],
                                    w_sb[kp][:, wi, :, m * P:(m + 1) * P],
                                    xt[kp][:, xi], start=st, stop=sp,
                                    perf_mode=DR)
                            for t in range(2):
                                nc.tensor.matmul(
                                    V01[t][:, 0:2 * P],
                                    xt[kp][:, xi, :, t * P:(t + 1) * P],
                                    w_sb[kp][:, wi, :, 4 * P:6 * P],
                                    start=st, stop=sp, perf_mode=DR)
                        if kp == 5 and mid is not None:
                            mid()
                        yield
                    # drain V first (V2/V3 sweeps wait on these PSUM
                    # banks), then QK (next block's S tiles are far off)
                    csl = slice(nb * NB, (nb + 1) * NB)
                    for t in range(2):
                        copy_on(o["qkv_v"][t % len(o["qkv_v"])],
                                Vp[nb * 4 + t][:], V01[t][:, 0:2 * P])
                    copy_on(o["qkv_q"], QTp[0][:, csl], S0[:, 0, :])
                    copy_on(o["qkv_k"], KTp[0][:, csl], S0[:, 1, :])
                    copy_on(o["qkv_q"], QTp[1][:, csl], S1[:, 0, :])
                    copy_on(o["qkv_k"], KTp[1][:, csl], S1[:, 1, :])
                    yield
                    # V2 then V3 sequentially: each holds only ONE O slot,
                    # so attention(qb0) can interleave using the other slot
                    for t in range(2, 4):
                        Vt = ps_tile("O", [P, NB], f"p1v{t}_{b}_{nb}",
                                     o["o_bufs"])
                        for kp in range(KP):
                            for ti, (wi, xi) in enumerate(TERMS):
                                nc.tensor.matmul(
                                    Vt[:, 0:2 * P],
                                    xt[kp][:, xi, :, t * P:(t + 1) * P],
                                    w_sb[kp][:, wi, :, 4 * P:6 * P],
                                    start=(kp == 0 and ti == 0),
                                    stop=(kp == KP - 1 and ti == 2),
                                    perf_mode=DR)
                            if kp % 2 == 1:
                                yield
                        copy_on(o["qkv_v"][t % len(o["qkv_v"])],
                                Vp[nb * 4 + t][:], Vt[:, 0:2 * P])
                        yield

                # ============ attention generator (one head) ============
                def attention_gen(b, h, qb):
                    """Yields once per kt-pair.  Returns the deferred
                    close-out thunk (broadcast + normalize + fp8 split)."""
                    nkt = 4 * qb + 4
                    qsl0 = qb * NB
                    ps_o = ps_tile("O", [P, NB], f"pso_{b}_{h}_{qb}", o["o_bufs"])
                    # two running-sum chains: even pairs / odd pairs
                    cdt = BF16 if o["chain_bf16"] else FP32
                    exs = [apool.tile([P, NB], cdt, name=f"exs{i}_{b}_{h}_{qb}",
                                      tag=f"exsum{i}", bufs=2) for i in range(2)]
                    npair = nkt // 2

                    def emit_scores_exp(p):
                        a, c = 2 * p, 2 * p + 1
                        oa, oc = NB - _width(a, qb), NB - _width(c, qb)
                        ps_s = ps_tile("S", [P, 2, NB], f"pss_{b}_{h}_{qb}_{p}",
                                       o["s_bufs"])
                        nc.tensor.matmul(ps_s[:, 0, oa:NB],
                                         KTp[h][:, a * P:(a + 1) * P],
                                         QTp[h][:, qsl0 + oa:qsl0 + NB],
                                         start=True, stop=True)
                        nc.tensor.matmul(ps_s[:, 1, oc:NB],
                                         KTp[h][:, c * P:(c + 1) * P],
                                         QTp[h][:, qsl0 + oc:qsl0 + NB],
                                         start=True, stop=True)
                        ex = apool.tile([P, 2, NB], BF16,
                                        name=f"ex_{b}_{h}_{qb}_{p}",
                                        tag="ex", bufs=o["ex_bufs"])
                        if oa == oc:
                            nc.scalar.activation(ex[:, :, oa:NB],
                                                 ps_s[:, :, oa:NB],
                                                 mybir.ActivationFunctionType.Exp,
                                                 scale=ESCALE)
                        else:
                            nc.scalar.activation(ex[:, 0, oa:NB],
                                                 ps_s[:, 0, oa:NB],
                                                 mybir.ActivationFunctionType.Exp,
                                                 scale=ESCALE)
                            nc.scalar.activation(ex[:, 1, oc:NB],
                                                 ps_s[:, 1, oc:NB],
                                                 mybir.ActivationFunctionType.Exp,
                                                 scale=ESCALE)
                        return (p, ex, oa, oc)

                    def emit_post(st):
                        p, ex, oa, oc = st
                        a, c = 2 * p, 2 * p + 1
                        # zero dead prefixes of narrowed (diagonal) tiles
                        if oa > 0:
                            nc.gpsimd.memset(ex[:, 0, 0:oa], 0.0)
                        if oc > 0:
                            nc.gpsimd.memset(ex[:, 1, 0:oc], 0.0)
                        # triangle masks on diagonal tiles
                        for half, kt, off in ((0, a, oa), (1, c, oc)):
                            if kt >= 4 * qb:
                                sl = ex[:, half, off:off + P]
                                if o["mask_eng"] == "P":
                                    nc.gpsimd.tensor_mul(sl, sl, tri[:])
                                else:
                                    nc.vector.tensor_mul(sl, sl, tri[:])
                        # denominator partial: exs[p%2] += ex.lo + ex.hi
                        tpr = apool.tile([P, NB], BF16,
                                         name=f"tp_{b}_{h}_{qb}_{p}",
                                         tag="tpr", bufs=2)
                        getattr(nc, _ENG_MAP[o["tpr_eng"]]).tensor_add(
                            tpr[:], ex[:, 0, :], ex[:, 1, :])
                        cp = o["chain_pat"][qb] if isinstance(o["chain_pat"], dict) else o["chain_pat"]
                        eng = getattr(nc, _ENG_MAP[cp[p % 2]])
                        if p < 2:
                            eng.tensor_copy(exs[p % 2][:], tpr[:])
                        else:
                            eng.tensor_add(exs[p % 2][:], exs[p % 2][:], tpr[:])
                        # AV accumulation
                        nc.tensor.matmul(ps_o[:, oa:NB],
                                         Vp[a][:, h * P:(h + 1) * P],
                                         ex[:, 0, oa:NB],
                                         start=(p == 0), stop=False,
                                         skip_group_check=True)
                        nc.tensor.matmul(ps_o[:, oc:NB],
                                         Vp[c][:, h * P:(h + 1) * P],
                                         ex[:, 1, oc:NB],
                                         start=False, stop=(p == npair - 1),
                                         skip_group_check=True)

                    # software pipeline: scores/exp of p+1 before AV of p
                    if o["pipe"]:
                        st = emit_scores_exp(0)
                        for p in range(npair):
                            nxt = (emit_scores_exp(p + 1)
                                   if p + 1 < npair else None)
                            emit_post(st)
                            st = nxt
                            yield
                    else:
                        for p in range(npair):
                            emit_post(emit_scores_exp(p))
                            yield
                    # denominator: merge chains, cross-partition reduce,
                    # recip; alrd=True folds reduce+broadcast into one
                    # gpsimd partition_all_reduce
                    mrg = apool.tile([P, NB], BF16, name=f"mg_{b}_{h}_{qb}",
                                     tag="mrg", bufs=2)
                    # final-batch qb3: both heads take the short bf16 tail
                    finq = b == B - 1 and qb == QB - 1
                    fin = finq and h == 1
                    qsls = ([slice(q * P, (q + 1) * P) for q in range(4)]
                            if fin else [slice(0, NB)])
                    if o["alrd"]:
                        dnb = apool.tile([P, NB], FP32, name=f"dn_{b}_{h}_{qb}",
                                         tag="dn", bufs=2)
                        rec = apool.tile([P, NB], F32R,
                                         name=f"rec_{b}_{h}_{qb}",
                                         tag="rec", bufs=2)
                        with nc.allow_low_precision(reason="f32r recip"):
                            for cs in qsls:
                                nc.vector.tensor_add(mrg[:, cs],
                                                     exs[0][:, cs],
                                                     exs[1][:, cs])
                                nc.gpsimd.partition_all_reduce(
                                    dnb[:, cs], mrg[:, cs], channels=P,
                                    reduce_op=bass_isa.ReduceOp.add)
                                nc.vector.reciprocal(rec[:, cs], dnb[:, cs])
                    else:
                        dn = apool.tile([1, NB], FP32, name=f"dn_{b}_{h}_{qb}",
                                        tag="dn", bufs=2)
                        rc1 = apool.tile([1, NB], F32R,
                                         name=f"rc_{b}_{h}_{qb}",
                                         tag="rc1", bufs=2)
                        with nc.allow_low_precision(reason="f32r recip"):
                            for cs in qsls:
                                nc.vector.tensor_add(mrg[:, cs],
                                                     exs[0][:, cs],
                                                     exs[1][:, cs])
                                nc.gpsimd.tensor_reduce(
                                    dn[:1, cs], mrg[:, cs],
                                    axis=mybir.AxisListType.C,
                                    op=mybir.AluOpType.add)
                                nc.vector.reciprocal(rc1[:1, cs],
                                                     dn[:1, cs])

                    def bc_of(cs):
                        if o["alrd"]:
                            return rec[:, cs]
                        bc = apool.tile([P, NB], F32R, name=f"bc_{b}_{h}_{qb}",
                                        tag="bc", bufs=2)
                        nc.gpsimd.partition_broadcast(bc[:, cs], rc1[:1, cs])
                        return bc[:, cs]

                    def close(q=None):
                        if finq:
                            # bf16 tail: outTb = ps_o * bc, no fp8 split
                            slices = ((slice(0, NB),) if q is None
                                      else (slice(q * P, (q + 1) * P),))
                            for cs in slices:
                                nc.vector.tensor_mul(outTb[:, h, cs],
                                                     ps_o[:, cs], bc_of(cs))
                            return
                        t1 = apool.tile([P, NB], BF16, name=f"t1_{b}_{h}_{qb}",
                                        tag="t1", bufs=2)
                        cs = slice(0, NB)
                        osl = slice(qsl0, qsl0 + NB)
                        nc.vector.tensor_mul(t1[:, cs], ps_o[:, cs], bc_of(cs))
                        copy_on(o["oh_eng"], outH[:, h, osl], t1[:, cs])
                        nc.vector.tensor_sub(outL[:, h, osl], t1[:, cs],
                                             outH[:, h, osl])
                    return close

                # ============ proj units ============
                yts = {}

                def proj_unit(b, tt, eb, eng_code, narrow_dma=False,
                              tag="Y"):
                    ps_y = ps_tile(tag, [P, NB], f"psy_{b}_{tt}_{eb}",
                                   o["y_bufs"])
                    tsl = slice(tt * P, (tt + 1) * P)
                    esl = slice(eb * NB, (eb + 1) * NB)
                    if b == B - 1 and tt >= 12:
                        # bf16 tail path (outTb holds qb3 tokens)
                        bsl = slice((tt - 12) * P, (tt - 11) * P)
                        nc.tensor.matmul(ps_y[:], outTb[:, 0, bsl],
                                         wobf_sb[0][:, esl],
                                         start=True, stop=False)
                        nc.tensor.matmul(ps_y[:], outTb[:, 1, bsl],
                                         wobf_sb[1][:, esl],
                                         start=False, stop=True)
                    else:
                        nc.tensor.matmul(ps_y[:], outH[:, :, tsl],
                                         wo_sb[:, 0, :, esl],
                                         start=True, stop=False, perf_mode=DR)
                        nc.tensor.matmul(ps_y[:], outL[:, :, tsl],
                                         wo_sb[:, 0, :, esl],
                                         start=False, stop=False, perf_mode=DR)
                        nc.tensor.matmul(ps_y[:], outH[:, :, tsl],
                                         wo_sb[:, 1, :, esl],
                                         start=False, stop=True, perf_mode=DR)
                    if eb == 0:
                        yts[(b, tt)] = apool.tile([P, QB, NB], BF16,
                                                  name=f"yt_{b}_{tt}",
                                                  tag="yt", bufs=o["yt_bufs"])
                    yt = yts[(b, tt)]
                    copy_on(eng_code, yt[:, eb, :], ps_y[:])
                    if narrow_dma:
                        # tail: stream each e-block out as soon as copied
                        nc.sync.dma_start(
                            y[b * T + tt * P:b * T + (tt + 1) * P,
                              eb * NB:(eb + 1) * NB], yt[:, eb, :])
                    elif b == B - 1 and tt >= 12 and eb in (1, QB - 1):
                        # final tiles: stream column-halves so the last DMA
                        # after the last matmul is half-sized
                        hsl = slice(0, 2) if eb == 1 else slice(2, QB)
                        nc.sync.dma_start(
                            y[b * T + tt * P:b * T + (tt + 1) * P,
                              hsl.start * NB:hsl.stop * NB], yt[:, hsl, :])
                    elif eb == QB - 1 and not (b == B - 1 and tt >= 12):
                        nc.sync.dma_start(
                            y[b * T + tt * P:b * T + (tt + 1) * P, :], yt[:])

                def proj_thunks(b, pqb, pat, narrow_dma=False, tags="Y"):
                    th = []
                    i = 0
                    for tt in range(4 * pqb, 4 * pqb + 4):
                        for eb in range(QB):
                            code = pat[i % len(pat)]
                            tag = tags[i % len(tags)]
                            th.append(lambda b=b, tt=tt, eb=eb, code=code,
                                      tag=tag:
                                      proj_unit(b, tt, eb, code, narrow_dma,
                                                tag))
                            i += 1
                    return th

                # ============ weaving driver ============
                def weave(gen, fillers, carry, defer=None):
                    """Run gen; after each yield emit carry thunks (once,
                    after o['defer_pairs'] yields) and a fair share of
                    fillers (popped from the shared list)."""
                    n = 0
                    held = 0
                    try:
                        while True:
                            next(gen)
                            n += 1
                            if n >= o["defer_pairs"] and carry:
                                for fn in carry:
                                    fn()
                                carry = []
                            if defer:
                                held += defer.pop(0)
                            if not carry:
                                while held > 0 and fillers:
                                    fillers.pop(0)()
                                    held -= 1
                    except StopIteration as si:
                        for fn in carry:
                            fn()
                        return si.value

                def share(nfill, nsteps):
                    base, rem = divmod(nfill, nsteps)
                    return [base + (1 if i < rem else 0) for i in range(nsteps)]

                def attention_unit(b, qb, fillers, carry, reserve=None,
                                   final=False):
                    if reserve is None:
                        reserve = o["reserve"]
                    npair = 2 * qb + 2
                    # hold a few fillers back to cover the close-out chain
                    # latency after the last AV pair
                    nres = min(reserve, len(fillers))
                    sh = share(len(fillers) - nres, 2 * npair)
                    close0 = weave(attention_gen(b, 0, qb), fillers, carry,
                                   defer=sh[:npair])
                    close1 = weave(attention_gen(b, 1, qb), fillers, [close0],
                                   defer=sh[npair:])
                    for fn in fillers:  # reserved + leftovers
                        fn()
                    del fillers[:]
                    return [close1]

                def prefix(gen, n):
                    for _ in range(n):
                        next(gen)
                        yield

                # ============ main schedule ============
                carry = []
                xts = {}
                for b in range(B):
                    # ---- phase 1 (+ second half of prev batch qb3 proj) ----
                    if b > 0:
                        ph1_fill = proj_thunks(b - 1, 3, o["pat_p1"])[8:]
                        qb0_fill = proj_thunks(b - 1, 3, o["pat_attn"][3])[:8]
                    else:
                        ph1_fill, qb0_fill = [], []
                    if b == 0:
                        xts["cur"] = load_nb(0, 0)
                    for nb in range(QB - 1):
                        nxt = [b, nb + 1]
                        mid = (lambda nxt=nxt:
                               xts.__setitem__("next", load_nb(*nxt)))
                        weave(phase1_nb(b, nb, xts["cur"], mid), ph1_fill,
                              carry,
                              defer=share(2, KP + 1) if ph1_fill else None)
                        xts["cur"] = xts["next"]
                        carry = []
                    # nb3: QK sweep + V01; the V2/V3 tail becomes PE filler
                    # for the attention(qb0) unit
                    mid = ((lambda: xts.__setitem__("next", load_nb(b + 1, 0)))
                           if b + 1 < B else None)
                    g_ph = phase1_nb(b, 3, xts["cur"], mid)
                    weave(prefix(g_ph, 9), ph1_fill, carry,
                          defer=share(2, 10) if ph1_fill else None)
                    if b + 1 < B:
                        xts["cur"] = xts["next"]
                    carry = []
                    for fn in ph1_fill:
                        fn()
                    tails = [(lambda: next(g_ph, None)) for _ in range(10)]
                    # ---- qb0 unit merged with phase-1 V tail ----
                    # one V2 step first fixes the O-slot rotation so the V3
                    # sweep can safely cover the h0 close-out chain
                    tails.pop(0)()
                    close0 = weave(attention_gen(b, 0, 0), tails, carry,
                                   defer=[2, 2])
                    for _ in range(3):  # V3 progress covers the dn/recip chain
                        if tails:
                            tails.pop(0)()
                    close0()
                    fill2 = tails + qb0_fill
                    close1 = weave(attention_gen(b, 1, 0), fill2,
                                   [], defer=[2, 2])
                    for fn in fill2:
                        fn()
                    carry = [close1]
                    carry = attention_unit(
                        b, 1, proj_thunks(b, 0, o["pat_attn"][0]), carry)
                    carry = attention_unit(
                        b, 2, proj_thunks(b, 1, o["pat_attn"][1]), carry)
                    carry = attention_unit(
                        b, 3, proj_thunks(b, 2, o["pat_attn"][2]), carry,
                        reserve=o["reserve3"])
                # ---- tail: final qb3 close quarters interleaved with the
                # bf16 tail proj units (token tile q unblocks on quarter q)
                close1 = carry[0]
                tail_th = proj_thunks(B - 1, 3, o["pat_tail"],
                                      narrow_dma=False, tags=o["tail_tags"])
                for q in range(4):
                    close1(q)
                    for fn in tail_th[4 * q:4 * q + 4]:
                        fn()
                if debug:
                    for h in range(HPC):
                        nc.sync.dma_start(dbg["QT"][h], QTp[h][:])
                        nc.sync.dma_start(dbg["KT"][h], KTp[h][:])
                    nc.sync.dma_start(dbg["outH"][:, :, :], outH[:])
                    nc.sync.dma_start(dbg["outL"][:, :, :], outL[:])
                    for kt in range(NT):
                        nc.sync.dma_start(dbg["V"][kt], Vp[kt][:])

    nc.compile()
    return nc


def _split8(a):
    """fp8 e4m3 hi + residual lo (scale-free: lo rides on subnormals)."""
    f8 = mybir.dt.np(F8)
    hi = a.astype(f8)
    lo = (a - hi.astype(np.float32)).astype(f8)
    return hi, lo


def prep_inputs(x, w_qkv, w_o):
    """Host-side shard prep. Returns per-core input maps (fp8 hi/lo)."""
    bf = mybir.dt.np(BF16)
    x = np.asarray(x, dtype=np.float32).reshape(B * T, D)
    xT = np.ascontiguousarray(x.T).reshape(KC, P, B * T)
    # [2hl, KC, P, BT] -> [KP, P, 2hl, 2ch, BT]
    x8 = np.stack(_split8(xT)).reshape(2, KP, 2, P, B * T)
    x8 = np.ascontiguousarray(x8.transpose(1, 3, 0, 2, 4))
    w_qkv = np.asarray(w_qkv, dtype=np.float32) * WSCALE
    w_o = np.asarray(w_o, dtype=np.float32) * WSCALE

    tri = np.zeros((P, P), dtype=np.float32)
    kp = np.arange(P)[:, None]
    qu = np.arange(P)[None, :]
    tri[kp <= qu] = 1.0
    tri = tri.astype(bf)
    onr = np.full((1, P), 1.0, dtype=np.float32)

    in_maps = []
    for c in range(NCORES):
        h0, h1 = HPC * c, HPC * c + 1
        cols = []
        for h in (h0, h1):
            cols += [w_qkv[h * DK:(h + 1) * DK],            # Q rows
                     w_qkv[D + h * DK:D + (h + 1) * DK]]    # K rows
        # reorder to Q0 K0 Q1 K1 then V0 V1
        cols = [cols[0], cols[1], cols[2], cols[3],
                w_qkv[2 * D + h0 * DK:2 * D + (h0 + 1) * DK] * (VSCALE / WSCALE),
                w_qkv[2 * D + h1 * DK:2 * D + (h1 + 1) * DK] * (VSCALE / WSCALE)]
        w = np.ascontiguousarray(
            np.concatenate(cols, 0).T).reshape(KC, P, WC)
        w8 = np.stack(_split8(w)).reshape(2, KP, 2, P, WC)
        w8 = np.ascontiguousarray(w8.transpose(1, 3, 0, 2, 4))
        # wo: [dk, head, outcol] from w_o[:, core cols].T [256, D]
        woT = np.ascontiguousarray(
            w_o[:, HPC * DK * c:HPC * DK * (c + 1)].T)
        wo = np.ascontiguousarray(
            woT.reshape(HPC, DK, D).transpose(1, 0, 2))
        wo8 = np.ascontiguousarray(np.stack(_split8(wo), axis=1))
        wobf = np.ascontiguousarray(woT.reshape(HPC, DK, D)).astype(bf)
        in_maps.append({
            "x8": x8, "w8": w8, "wo8": wo8, "wobf": wobf,
            "tri": tri, "onr": onr,
        })
    return in_maps


_nc_cache = {}


def get_nc(debug=False, **opts):
    key = (debug, tuple(sorted((k, str(v)) for k, v in opts.items())))
    if key not in _nc_cache:
        _nc_cache[key] = build(debug=debug, **opts)
    return _nc_cache[key]


def run(x, w_qkv, w_o, debug=False, **opts):
    nc = get_nc(debug=debug, **opts)
    in_maps = prep_inputs(x, w_qkv, w_o)
    res = bass_utils.run_bass_kernel_spmd(nc, in_maps, core_ids=list(range(NCORES)))
    return res


def kernel(x, w_qkv, w_o):
    res = run(x, w_qkv, w_o)
    y = res.results[0]["y"].astype(np.float64)
    for c in range(1, NCORES):
        y += res.results[c]["y"]
    return (y / YDIV).astype(np.float32).reshape(B, T, D)


# revision 23
# speedup vs baseline: 1.1567x; 1.0019x over previous
"""Trainium2 Bass kernel for causal multi-head attention (dense transformer block).

Math (reference semantics):
    qkv = x @ w_qkv.T ; split into Q,K,V heads [B,H,T,dk]
    (rotary in the reference rotates Q and K of head h by a constant,
     time-independent orthogonal rotation R_h; since scores = (R_h q)·(R_h k)
     = q·k, the rotation cancels exactly and is skipped here)
    scores = causal_mask(Q @ K.T / sqrt(dk)); attn = softmax(scores)
    out = attn @ V ; y = out @ w_o.T

Sharding: head-parallel over 8 cores (2 heads/core, both batches).  Each core
computes a partial y (its heads' contribution through w_o columns); the host
sums the 8 partials (the "all-reduce").

v4 design (vs the bf16 v3 baseline):
  * QKV projection and out-projection run as 3-term compensated fp8
    (e4m3) DoubleRow matmuls: operands are split hi+lo (lo is the
    quantization residual, representable via fp8 subnormals without any
    rescale), and x@w = xh@wh + xl@wh + xh@wl accumulates in fp32 PSUM.
    DoubleRow contracts 2 k-tiles (256) per instruction at 0.5 cyc/row,
    so the three terms cost 0.75x the bf16 schedule on PE while being
    MORE accurate than bf16 (the residual kills the quantization error).
  * hi and lo planes are packed in one dram tensor so each chunk-pair /
    weight tile still costs ONE hwdge descriptor-generation slot (the
    dge fixed cost, not bytes, is the phase-1 DMA limiter).
  * Scale plumbing (no extra device ops): Q/K rows of w_qkv and all of
    w_o are scaled x64 host-side so their fp8 mantissas are in range
    (the exp scale absorbs the resulting x4096 on scores); V rows are
    scaled x32 so outT = ps_o * (1/denom) lands at x32 with good fp8
    hi/lo precision; y carries x32x64 and the host divides by 2048.
  * Attention (scores, exp, AV) stays bf16: plain-fp8 Q/K/ex/V all
    overshoot the error budget and compensated fp8 is slower than bf16
    at dk=128 contraction.
  * Softmax close-out: GpSimd partition_all_reduce folds the
    cross-partition denominator sum and its broadcast into one op, so
    DVE reciprocal directly yields the [P,NB] normalize multiplier (no
    PE ones-matmul); the normalized outT is written as fp8 hi + fp8
    residual lo feeding the DoubleRow out-projection.
  * DMA rule (hard-won): dma_start pairs elements by flat AP iteration
    order, NOT by dimension names.  Every dram tensor here is laid out
    so a DMA slice iterates exactly like its destination SBUF tile
    ([kp, p, hi/lo, parity, cols]); getting this wrong scrambles rows
    silently (the all-hi term still "works" because both operands get
    the same scramble -- only cross terms and the out-proj break).
  * Tail: the last batch's qb3 close-outs write bf16 outT directly
    (short chain, no fp8 split), quartered by token columns, and the
    final 16 proj units run bf16 2-matmul style interleaved with the
    quarters so PE restarts ~3us earlier.
  * Schedule (weaving of proj units into attention kt-pairs, deferred
    close-out chains, chunk-pair DMAs, causal narrowing with GpSimd
    memset/triangle masks) is inherited from v3.
"""

import contextlib

import numpy as np

import concourse.bacc as bacc
import concourse.bass as bass
import concourse.bass_isa as bass_isa
import concourse.mybir as mybir
import concourse.tile as tile
from concourse import bass_utils

B, T, D, H, DK = 2, 2048, 2048, 16, 128
NCORES = 8
HPC = H // NCORES  # heads per core
P = 128
NB = 512           # q-block / token-block / e-block width
KC = D // P        # 16 contraction chunks of the model dim
KP = KC // 2       # chunk pairs
QB = T // NB       # 4 q blocks per batch
NT = T // P        # 16 token tiles per batch
WC = 6 * P         # w columns per chunk: Q0 K0 Q1 K1 V0 V1
FP32 = mybir.dt.float32
F32R = mybir.dt.float32r
BF16 = mybir.dt.bfloat16
F8 = mybir.dt.float8e4
DR = mybir.MatmulPerfMode.DoubleRow
WSCALE = 64.0      # host scale on w_qkv Q/K rows and w_o (fp8 range)
VSCALE = 32.0      # host scale on w_qkv V rows => outT carries x32
YDIV = VSCALE * WSCALE  # host divides gathered y by this
SCALE = 1.0 / np.sqrt(DK)
ESCALE = SCALE / (WSCALE * WSCALE)  # exp() input scale: Q,K carry x64 each

DEFAULT_OPTS = dict(
    ex_bufs=6, xt_bufs=2, yt_bufs=4, s_bufs=2, o_bufs=2, y_bufs=2,
    loop_n=1,
    # yt-copy engine cycle per *hosting location* of the proj units
    pat_attn={0: "DAA", 1: "DA", 2: "DA", 3: "DAA"},
    pat_p1="DA", pat_tail="AD",
    defer_pairs=4,
    chain_pat={0: "DD", 1: "PD", 2: "PD", 3: "PD"},  # per qb (even, odd pair)
    mask_eng="D",
    qkv_q="A", qkv_k="D", qkv_v="DD", pipe=True, chain_bf16=True,
    oh_eng="D",     # outH fp8 copy engine
    tpr_eng="D",    # per-pair ex.lo+ex.hi add engine
    alrd=True,      # partition_all_reduce denominator (no broadcast stage)
    shift2=0,       # proj thunks moved from the qb1 weave into qb2's
    shift3=0,       # proj thunks moved from the qb2 weave into qb3's
    reserve3=6,     # fillers reserved past the final attention weave
    reserve=3,      # fillers reserved in non-final attention units
    tail_tags="YO",
)

_ENG_MAP = {"P": "gpsimd", "A": "scalar", "D": "vector"}


def _width(kt, qb):
    """Live q-suffix width of key tile kt within q-block qb (causal)."""
    j = kt - 4 * qb
    if j <= 0:
        return NB
    return NB - P * j


def build(debug=False, **opts):
    o = dict(DEFAULT_OPTS)
    o.update({k: v for k, v in opts.items() if k in DEFAULT_OPTS})
    nc = bacc.Bacc("TRN2", target_bir_lowering=False, debug=False,
                   num_devices=NCORES)
    # 3D dram layouts allow one DMA per chunk-pair / token tile; dim 0
    # packs the fp8 hi/lo planes so one DMA moves both.
    # dim order matches the SBUF tile iteration order exactly (dma_start
    # pairs elements by flat AP order): [kp, p, hi/lo, chunk-parity, cols]
    x8 = nc.dram_tensor("x8", [KP, P, 2, 2, B * T], F8, kind="ExternalInput")
    w8 = nc.dram_tensor("w8", [KP, P, 2, 2, WC], F8, kind="ExternalInput")
    wo8 = nc.dram_tensor("wo8", [P, 2, HPC, D], F8, kind="ExternalInput")
    wobf_d = nc.dram_tensor("wobf", [HPC, P, D], BF16, kind="ExternalInput")
    tri_d = nc.dram_tensor("tri", [P, P], BF16, kind="ExternalInput")
    onr_d = nc.dram_tensor("onr", [1, P], F32R, kind="ExternalInput")
    y = nc.dram_tensor("y", [B * T, D], BF16, kind="ExternalOutput")
    dbg = {}
    if debug:
        for nm in ("QT", "KT"):
            dbg[nm] = nc.dram_tensor(f"dbg_{nm}", [HPC, P, T], BF16,
                                     kind="ExternalOutput")
        dbg["outH"] = nc.dram_tensor("dbg_outH", [P, HPC, T], F8,
                                     kind="ExternalOutput")
        dbg["outL"] = nc.dram_tensor("dbg_outL", [P, HPC, T], F8,
                                     kind="ExternalOutput")
        dbg["V"] = nc.dram_tensor("dbg_V", [NT, P, 2 * P], BF16,
                                  kind="ExternalOutput")

    with tile.TileContext(nc) as tc:
        with (
            tc.tile_pool(name="const", bufs=1) as cpool,
            tc.tile_pool(name="xp", bufs=1) as xpool,
            tc.tile_pool(name="qkv", bufs=1) as qpool,
            tc.tile_pool(name="attn", bufs=1) as apool,
            tc.tile_pool(name="ps", bufs=1, space="PSUM") as pspool,
        ):
            # ---- constants / weights resident in SBUF ----
            # dim1 = hi/lo plane, dim2 = chunk parity (DoubleRow k-tiles)
            w_sb = [cpool.tile([P, 2, 2, WC], F8, name=f"w_{kp}")
                    for kp in range(KP)]
            wo_sb = cpool.tile([P, 2, HPC, D], F8, name="wo")
            wobf_sb = [cpool.tile([P, D], BF16, name=f"wobf{h}")
                       for h in range(HPC)]
            tri = cpool.tile([P, P], BF16, name="tri")
            onr = cpool.tile([1, P], F32R, name="onr")

            # persistent per-batch state (WAR deps recycle across batches)
            QTp = [qpool.tile([P, T], BF16, name=f"QT{h}") for h in range(HPC)]
            KTp = [qpool.tile([P, T], BF16, name=f"KT{h}") for h in range(HPC)]
            Vp = [qpool.tile([P, 2 * P], BF16, name=f"V{kt}") for kt in range(NT)]
            outH = qpool.tile([P, HPC, T], F8, name="outH")
            outL = qpool.tile([P, HPC, T], F8, name="outL")
            outTb = qpool.tile([P, HPC, NB], BF16, name="outTb")  # tail qb3

            def ps_tile(tag, shape, name, bufs):
                return pspool.tile(shape, FP32, name=name, tag=tag, bufs=bufs)

            def copy_on(code, dst, src):
                eng = _ENG_MAP[code]
                if eng == "gpsimd":
                    nc.gpsimd.tensor_copy(dst, src)
                elif eng == "scalar":
                    nc.scalar.copy(dst, src)
                else:
                    nc.vector.tensor_copy(dst, src)

            loop_ctx = (tc.For_i(0, o["loop_n"], 1, hint_engines=(
                            mybir.EngineType.PE, mybir.EngineType.Activation,
                            mybir.EngineType.DVE, mybir.EngineType.SP,
                            mybir.EngineType.Pool))
                        if o["loop_n"] > 1 else contextlib.nullcontext())

            if o["loop_n"] > 1:
                # weights/constants loaded once, outside the HW loop
                for kp in range(KP):
                    nc.sync.dma_start(w_sb[kp][:], w8[kp])
                nc.sync.dma_start(tri[:], tri_d[:, :])
                nc.sync.dma_start(onr[:], onr_d[:, :])
                nc.sync.dma_start(wo_sb[:], wo8[:, :, :, :])
                for h in range(HPC):
                    nc.sync.dma_start(wobf_sb[h][:], wobf_d[h])

            with loop_ctx:
                # ============ phase 1 generator (one token block) ============
                def load_nb(b, nb):
                    """Allocate + DMA the x tiles for token block (b, nb)."""
                    col0 = b * T + nb * NB
                    xt = [xpool.tile([P, 2, 2, NB], F8, name=f"x{kp}_{b}_{nb}",
                                     tag=f"x{kp}", bufs=o["xt_bufs"])
                          for kp in range(KP)]
                    first = b == 0 and nb == 0 and o["loop_n"] == 1
                    for kp in range(KP):
                        if first and kp == 0:
                            # split the first loads so the first Q/K matmuls
                            # unblock after the hi Q0K0 columns + hi x0
                            nc.sync.dma_start(w_sb[0][:, 0, :, 0:2 * P],
                                              w8[0, :, 0, :, 0:2 * P])
                            nc.sync.dma_start(
                                xt[0][:, 0],
                                x8[0, :, 0, :, col0:col0 + NB])
                            nc.sync.dma_start(w_sb[0][:, 0, :, 2 * P:WC],
                                              w8[0, :, 0, :, 2 * P:WC])
                            nc.sync.dma_start(
                                xt[0][:, 1],
                                x8[0, :, 1, :, col0:col0 + NB])
                            nc.sync.dma_start(w_sb[0][:, 1], w8[0, :, 1])
                            continue
                        if first:
                            nc.sync.dma_start(w_sb[kp][:], w8[kp])
                        nc.sync.dma_start(
                            xt[kp][:], x8[kp, :, :, :, col0:col0 + NB])
                        if first and kp == 1:
                            nc.sync.dma_start(tri[:], tri_d[:, :])
                            nc.sync.dma_start(onr[:], onr_d[:, :])
                    return xt

                # (w_hl, x_hl) term order: hi@hi, hi-w@lo-x, lo-w@hi-x
                TERMS = ((0, 0), (0, 1), (1, 0))

                def phase1_nb(b, nb, xt, mid=None):
                    """QKV projection for token block (b, nb), k-outer,
                    3-term compensated fp8 DoubleRow.  Yields after each
                    chunk-pair (8) + drains; `mid` thunk (next-block
                    prefetch) fires after chunk-pair 5."""
                    if b == 0 and nb == 1 and o["loop_n"] == 1:
                        nc.sync.dma_start(wo_sb[:], wo8[:, :, :, :])
                    if b == 1 and nb == 1 and o["loop_n"] == 1:
                        for h in range(HPC):
                            nc.sync.dma_start(wobf_sb[h][:], wobf_d[h])

                    S0 = ps_tile("S", [P, 2, NB], f"p1s0_{b}_{nb}", o["s_bufs"])
                    S1 = ps_tile("S", [P, 2, NB], f"p1s1_{b}_{nb}", o["s_bufs"])
                    # V token-tile groups need a PSUM bank each (one
                    # accumulation group per bank): two sub-sweeps of 2.
                    V01 = [ps_tile("O", [P, NB], f"p1v{t}_{b}_{nb}", o["o_bufs"])
                           for t in range(2)]
                    SQK = (S0[:, 0, :], S0[:, 1, :], S1[:, 0, :], S1[:, 1, :])
                    first_nb = b == 0 and nb == 0 and o["loop_n"] == 1
                    for kp in range(KP):
                        for ti, (wi, xi) in enumerate(TERMS):
                            st = kp == 0 and ti == 0
                            sp = kp == KP - 1 and ti == 2
                            for m in range(4):
                                nc.tensor.matmul(
                                    SQK[m],
                                    w_sb[kp][:, wi, :, m * P:(m + 1) * P],
                                    xt[kp][:, xi], start=st, stop=sp,
                                    perf_mode=DR)
                            for t in range(2):
                                nc.tensor.matmul(
                                    V01[t][:, 0:2 * P],
                                    xt[kp][:, xi, :, t * P:(t + 1) * P],
                                    w_sb[kp][:, wi, :, 4 * P:6 * P],
                                    start=st, stop=sp, perf_mode=DR)
                        if kp == 5 and mid is not None:
                            mid()
                        yield
                    # drain V first (V2/V3 sweeps wait on these PSUM
                    # banks), then QK (next block's S tiles are far off)
                    csl = slice(nb * NB, (nb + 1) * NB)
                    for t in range(2):
                        copy_on(o["qkv_v"][t % len(o["qkv_v"])],
                                Vp[nb * 4 + t][:], V01[t][:, 0:2 * P])
                    copy_on(o["qkv_q"], QTp[0][:, csl], S0[:, 0, :])
                    copy_on(o["qkv_k"], KTp[0][:, csl], S0[:, 1, :])
                    copy_on(o["qkv_q"], QTp[1][:, csl], S1[:, 0, :])
                    copy_on(o["qkv_k"], KTp[1][:, csl], S1[:, 1, :])
                    yield
                    # V2 then V3 sequentially: each holds only ONE O slot,
                    # so attention(qb0) can interleave using the other slot
                    for t in range(2, 4):
                        Vt = ps_tile("O", [P, NB], f"p1v{t}_{b}_{nb}",
                                     o["o_bufs"])
                        for kp in range(KP):
                            for ti, (wi, xi) in enumerate(TERMS):
                                nc.tensor.matmul(
                                    Vt[:, 0:2 * P],
                                    xt[kp][:, xi, :, t * P:(t + 1) * P],
                                    w_sb[kp][:, wi, :, 4 * P:6 * P],
                                    start=(kp == 0 and ti == 0),
                                    stop=(kp == KP - 1 and ti == 2),
                                    perf_mode=DR)
                            if kp % 2 == 1:
                                yield
                        copy_on(o["qkv_v"][t % len(o["qkv_v"])],
                                Vp[nb * 4 + t][:], Vt[:, 0:2 * P])
                        yield

                # ============ attention generator (one head) ============
                def attention_gen(b, h, qb):
                    """Yields once per kt-pair.  Returns the deferred
                    close-out thunk (broadcast + normalize + fp8 split)."""
                    nkt = 4 * qb + 4
                    qsl0 = qb * NB
                    ps_o = ps_tile("O", [P, NB], f"pso_{b}_{h}_{qb}", o["o_bufs"])
                    # two running-sum chains: even pairs / odd pairs
                    cdt = BF16 if o["chain_bf16"] else FP32
                    exs = [apool.tile([P, NB], cdt, name=f"exs{i}_{b}_{h}_{qb}",
                                      tag=f"exsum{i}", bufs=2) for i in range(2)]
                    npair = nkt // 2

                    def emit_scores_exp(p):
                        a, c = 2 * p, 2 * p + 1
                        oa, oc = NB - _width(a, qb), NB - _width(c, qb)
                        ps_s = ps_tile("S", [P, 2, NB], f"pss_{b}_{h}_{qb}_{p}",
                                       o["s_bufs"])
                        nc.tensor.matmul(ps_s[:, 0, oa:NB],
                                         KTp[h][:, a * P:(a + 1) * P],
                                         QTp[h][:, qsl0 + oa:qsl0 + NB],
                                         start=True, stop=True)
                        nc.tensor.matmul(ps_s[:, 1, oc:NB],
                                         KTp[h][:, c * P:(c + 1) * P],
                                         QTp[h][:, qsl0 + oc:qsl0 + NB],
                                         start=True, stop=True)
                        ex = apool.tile([P, 2, NB], BF16,
                                        name=f"ex_{b}_{h}_{qb}_{p}",
                                        tag="ex", bufs=o["ex_bufs"])
                        if oa == oc:
                            nc.scalar.activation(ex[:, :, oa:NB],
                                                 ps_s[:, :, oa:NB],
                                                 mybir.ActivationFunctionType.Exp,
                                                 scale=ESCALE)
                        else:
                            nc.scalar.activation(ex[:, 0, oa:NB],
                                                 ps_s[:, 0, oa:NB],
                                                 mybir.ActivationFunctionType.Exp,
                                                 scale=ESCALE)
                            nc.scalar.activation(ex[:, 1, oc:NB],
                                                 ps_s[:, 1, oc:NB],
                                                 mybir.ActivationFunctionType.Exp,
                                                 scale=ESCALE)
                        return (p, ex, oa, oc)

                    def emit_post(st):
                        p, ex, oa, oc = st
                        a, c = 2 * p, 2 * p + 1
                        # zero dead prefixes of narrowed (diagonal) tiles
                        if oa > 0:
                            nc.gpsimd.memset(ex[:, 0, 0:oa], 0.0)
                        if oc > 0:
                            nc.gpsimd.memset(ex[:, 1, 0:oc], 0.0)
                        # triangle masks on diagonal tiles
                        for half, kt, off in ((0, a, oa), (1, c, oc)):
                            if kt >= 4 * qb:
                                sl = ex[:, half, off:off + P]
                                if o["mask_eng"] == "P":
                                    nc.gpsimd.tensor_mul(sl, sl, tri[:])
                                else:
                                    nc.vector.tensor_mul(sl, sl, tri[:])
                        # denominator partial: exs[p%2] += ex.lo + ex.hi
                        tpr = apool.tile([P, NB], BF16,
                                         name=f"tp_{b}_{h}_{qb}_{p}",
                                         tag="tpr", bufs=2)
                        getattr(nc, _ENG_MAP[o["tpr_eng"]]).tensor_add(
                            tpr[:], ex[:, 0, :], ex[:, 1, :])
                        cp = o["chain_pat"][qb] if isinstance(o["chain_pat"], dict) else o["chain_pat"]
                        eng = getattr(nc, _ENG_MAP[cp[p % 2]])
                        if p < 2:
                            eng.tensor_copy(exs[p % 2][:], tpr[:])
                        else:
                            eng.tensor_add(exs[p % 2][:], exs[p % 2][:], tpr[:])
                        # AV accumulation
                        nc.tensor.matmul(ps_o[:, oa:NB],
                                         Vp[a][:, h * P:(h + 1) * P],
                                         ex[:, 0, oa:NB],
                                         start=(p == 0), stop=False,
                                         skip_group_check=True)
                        nc.tensor.matmul(ps_o[:, oc:NB],
                                         Vp[c][:, h * P:(h + 1) * P],
                                         ex[:, 1, oc:NB],
                                         start=False, stop=(p == npair - 1),
                                         skip_group_check=True)

                    # denominator tiles (hoisted: the fin path emits its
                    # quarter chains inside the pair loop)
                    mrg = apool.tile([P, NB], BF16, name=f"mg_{b}_{h}_{qb}",
                                     tag="mrg", bufs=2)
                    # final-batch qb3: both heads take the short bf16 tail
                    finq = b == B - 1 and qb == QB - 1
                    fin = finq and h == 1
                    qsls = ([slice(q * P, (q + 1) * P) for q in range(4)]
                            if fin else [slice(0, NB)])
                    if o["alrd"]:
                        dnb = apool.tile([P, NB], FP32, name=f"dn_{b}_{h}_{qb}",
                                         tag="dn", bufs=2)
                        rec = apool.tile([P, NB], F32R,
                                         name=f"rec_{b}_{h}_{qb}",
                                         tag="rec", bufs=2)

                        def emit_chain_qs(qs):
                            with nc.allow_low_precision(reason="f32r recip"):
                                for q in qs:
                                    cs = slice(q * P, (q + 1) * P)
                                    nc.vector.tensor_add(mrg[:, cs],
                                                         exs[0][:, cs],
                                                         exs[1][:, cs])
                                    nc.gpsimd.partition_all_reduce(
                                        dnb[:, cs], mrg[:, cs], channels=P,
                                        reduce_op=bass_isa.ReduceOp.add)
                                    nc.vector.reciprocal(rec[:, cs],
                                                         dnb[:, cs])
                        fin_chain = [emit_chain_qs]
                    else:
                        fin_chain = [None]

                    # software pipeline: scores/exp of p+1 before AV of p
                    if o["pipe"]:
                        st = emit_scores_exp(0)
                        for p in range(npair):
                            nxt = (emit_scores_exp(p + 1)
                                   if p + 1 < npair else None)
                            emit_post(st)
                            if fin and p == npair - 2:
                                # the last pair is dead in columns [0:2P):
                                # quarters q0/q1 of the denominator AND the
                                # outTb normalize close one pair early (exs
                                # chains and ps_o are complete there), so
                                # tail tiles tt12/tt13 unblock before the
                                # last pair retires
                                fin_chain[0]([0, 1])
                                for q in (0, 1):
                                    cq = slice(q * P, (q + 1) * P)
                                    nc.vector.tensor_mul(outTb[:, h, cq],
                                                         ps_o[:, cq],
                                                         rec[:, cq])
                            if fin and p == npair - 1:
                                fin_chain[0]([2, 3])
                                for q in (2, 3):
                                    cq = slice(q * P, (q + 1) * P)
                                    nc.vector.tensor_mul(outTb[:, h, cq],
                                                         ps_o[:, cq],
                                                         rec[:, cq])
                            st = nxt
                            yield
                    else:
                        for p in range(npair):
                            emit_post(emit_scores_exp(p))
                            yield
                    # denominator: merge chains, cross-partition reduce,
                    # recip; alrd=True folds reduce+broadcast into one
                    # gpsimd partition_all_reduce
                    if o["alrd"]:
                        if not fin:
                            with nc.allow_low_precision(reason="f32r recip"):
                                for cs in qsls:
                                    nc.vector.tensor_add(mrg[:, cs],
                                                         exs[0][:, cs],
                                                         exs[1][:, cs])
                                    nc.gpsimd.partition_all_reduce(
                                        dnb[:, cs], mrg[:, cs], channels=P,
                                        reduce_op=bass_isa.ReduceOp.add)
                                    nc.vector.reciprocal(rec[:, cs],
                                                         dnb[:, cs])
                    else:
                        dn = apool.tile([1, NB], FP32, name=f"dn_{b}_{h}_{qb}",
                                        tag="dn", bufs=2)
                        rc1 = apool.tile([1, NB], F32R,
                                         name=f"rc_{b}_{h}_{qb}",
                                         tag="rc1", bufs=2)
                        with nc.allow_low_precision(reason="f32r recip"):
                            for cs in qsls:
                                nc.vector.tensor_add(mrg[:, cs],
                                                     exs[0][:, cs],
                                                     exs[1][:, cs])
                                nc.gpsimd.tensor_reduce(
                                    dn[:1, cs], mrg[:, cs],
                                    axis=mybir.AxisListType.C,
                                    op=mybir.AluOpType.add)
                                nc.vector.reciprocal(rc1[:1, cs],
                                                     dn[:1, cs])

                    def bc_of(cs):
                        if o["alrd"]:
                            return rec[:, cs]
                        bc = apool.tile([P, NB], F32R, name=f"bc_{b}_{h}_{qb}",
                                        tag="bc", bufs=2)
                        nc.gpsimd.partition_broadcast(bc[:, cs], rc1[:1, cs])
                        return bc[:, cs]

                    def close(q=None):
                        if fin and o["alrd"]:
                            return  # emitted early inside the pair loop
                        if finq:
                            # bf16 tail: outTb = ps_o * bc, no fp8 split
                            slices = ((slice(0, NB),) if q is None
                                      else (slice(q * P, (q + 1) * P),))
                            for cs in slices:
                                nc.vector.tensor_mul(outTb[:, h, cs],
                                                     ps_o[:, cs], bc_of(cs))
                            return
                        t1 = apool.tile([P, NB], BF16, name=f"t1_{b}_{h}_{qb}",
                                        tag="t1", bufs=2)
                        cs = slice(0, NB)
                        osl = slice(qsl0, qsl0 + NB)
                        nc.vector.tensor_mul(t1[:, cs], ps_o[:, cs], bc_of(cs))
                        copy_on(o["oh_eng"], outH[:, h, osl], t1[:, cs])
                        nc.vector.tensor_sub(outL[:, h, osl], t1[:, cs],
                                             outH[:, h, osl])
                    return close

                # ============ proj units ============
                yts = {}

                def proj_unit(b, tt, eb, eng_code, narrow_dma=False,
                              tag="Y"):
                    ps_y = ps_tile(tag, [P, NB], f"psy_{b}_{tt}_{eb}",
                                   o["y_bufs"])
                    tsl = slice(tt * P, (tt + 1) * P)
                    esl = slice(eb * NB, (eb + 1) * NB)
                    if b == B - 1 and tt >= 12:
                        # bf16 tail path (outTb holds qb3 tokens)
                        bsl = slice((tt - 12) * P, (tt - 11) * P)
                        nc.tensor.matmul(ps_y[:], outTb[:, 0, bsl],
                                         wobf_sb[0][:, esl],
                                         start=True, stop=False)
                        nc.tensor.matmul(ps_y[:], outTb[:, 1, bsl],
                                         wobf_sb[1][:, esl],
                                         start=False, stop=True)
                    else:
                        nc.tensor.matmul(ps_y[:], outH[:, :, tsl],
                                         wo_sb[:, 0, :, esl],
                                         start=True, stop=False, perf_mode=DR)
                        nc.tensor.matmul(ps_y[:], outL[:, :, tsl],
                                         wo_sb[:, 0, :, esl],
                                         start=False, stop=False, perf_mode=DR)
                        nc.tensor.matmul(ps_y[:], outH[:, :, tsl],
                                         wo_sb[:, 1, :, esl],
                                         start=False, stop=True, perf_mode=DR)
                    if eb == 0:
                        yts[(b, tt)] = apool.tile([P, QB, NB], BF16,
                                                  name=f"yt_{b}_{tt}",
                                                  tag="yt", bufs=o["yt_bufs"])
                    yt = yts[(b, tt)]
                    copy_on(eng_code, yt[:, eb, :], ps_y[:])
                    if narrow_dma:
                        # tail: stream each e-block out as soon as copied
                        nc.sync.dma_start(
                            y[b * T + tt * P:b * T + (tt + 1) * P,
                              eb * NB:(eb + 1) * NB], yt[:, eb, :])
                    elif b == B - 1 and tt >= 12 and eb in (1, QB - 1):
                        # final tiles: stream column-halves so the last DMA
                        # after the last matmul is half-sized
                        hsl = slice(0, 2) if eb == 1 else slice(2, QB)
                        nc.sync.dma_start(
                            y[b * T + tt * P:b * T + (tt + 1) * P,
                              hsl.start * NB:hsl.stop * NB], yt[:, hsl, :])
                    elif eb == QB - 1 and not (b == B - 1 and tt >= 12):
                        nc.sync.dma_start(
                            y[b * T + tt * P:b * T + (tt + 1) * P, :], yt[:])

                def proj_thunks(b, pqb, pat, narrow_dma=False, tags="Y"):
                    th = []
                    i = 0
                    for tt in range(4 * pqb, 4 * pqb + 4):
                        for eb in range(QB):
                            code = pat[i % len(pat)]
                            tag = tags[i % len(tags)]
                            th.append(lambda b=b, tt=tt, eb=eb, code=code,
                                      tag=tag:
                                      proj_unit(b, tt, eb, code, narrow_dma,
                                                tag))
                            i += 1
                    return th

                # ============ weaving driver ============
                def weave(gen, fillers, carry, defer=None):
                    """Run gen; after each yield emit carry thunks (once,
                    after o['defer_pairs'] yields) and a fair share of
                    fillers (popped from the shared list)."""
                    n = 0
                    held = 0
                    try:
                        while True:
                            next(gen)
                            n += 1
                            if n >= o["defer_pairs"] and carry:
                                for fn in carry:
                                    fn()
                                carry = []
                            if defer:
                                held += defer.pop(0)
                            if not carry:
                                while held > 0 and fillers:
                                    fillers.pop(0)()
                                    held -= 1
                    except StopIteration as si:
                        for fn in carry:
                            fn()
                        return si.value

                def share(nfill, nsteps):
                    base, rem = divmod(nfill, nsteps)
                    return [base + (1 if i < rem else 0) for i in range(nsteps)]

                def attention_unit(b, qb, fillers, carry, reserve=None,
                                   final=False):
                    if reserve is None:
                        reserve = o["reserve"]
                    npair = 2 * qb + 2
                    # hold a few fillers back to cover the close-out chain
                    # latency after the last AV pair
                    nres = min(reserve, len(fillers))
                    sh = share(len(fillers) - nres, 2 * npair)
                    close0 = weave(attention_gen(b, 0, qb), fillers, carry,
                                   defer=sh[:npair])
                    close1 = weave(attention_gen(b, 1, qb), fillers, [close0],
                                   defer=sh[npair:])
                    for fn in fillers:  # reserved + leftovers
                        fn()
                    del fillers[:]
                    return [close1]

                def prefix(gen, n):
                    for _ in range(n):
                        next(gen)
                        yield

                # ============ main schedule ============
                carry = []
                xts = {}
                for b in range(B):
                    # ---- phase 1 (+ second half of prev batch qb3 proj) ----
                    if b > 0:
                        ph1_fill = proj_thunks(b - 1, 3, o["pat_p1"])[8:]
                        qb0_fill = proj_thunks(b - 1, 3, o["pat_attn"][3])[:8]
                    else:
                        ph1_fill, qb0_fill = [], []
                    if b == 0:
                        xts["cur"] = load_nb(0, 0)
                    for nb in range(QB - 1):
                        nxt = [b, nb + 1]
                        mid = (lambda nxt=nxt:
                               xts.__setitem__("next", load_nb(*nxt)))
                        weave(phase1_nb(b, nb, xts["cur"], mid), ph1_fill,
                              carry,
                              defer=share(2, KP + 1) if ph1_fill else None)
                        xts["cur"] = xts["next"]
                        carry = []
                    # nb3: QK sweep + V01; the V2/V3 tail becomes PE filler
                    # for the attention(qb0) unit
                    mid = ((lambda: xts.__setitem__("next", load_nb(b + 1, 0)))
                           if b + 1 < B else None)
                    g_ph = phase1_nb(b, 3, xts["cur"], mid)
                    weave(prefix(g_ph, 9), ph1_fill, carry,
                          defer=share(2, 10) if ph1_fill else None)
                    if b + 1 < B:
                        xts["cur"] = xts["next"]
                    carry = []
                    for fn in ph1_fill:
                        fn()
                    tails = [(lambda: next(g_ph, None)) for _ in range(10)]
                    # ---- qb0 unit merged with phase-1 V tail ----
                    # one V2 step first fixes the O-slot rotation so the V3
                    # sweep can safely cover the h0 close-out chain
                    tails.pop(0)()
                    close0 = weave(attention_gen(b, 0, 0), tails, carry,
                                   defer=[2, 2])
                    for _ in range(3):  # V3 progress covers the dn/recip chain
                        if tails:
                            tails.pop(0)()
                    close0()
                    fill2 = tails + qb0_fill
                    close1 = weave(attention_gen(b, 1, 0), fill2,
                                   [], defer=[2, 2])
                    for fn in fill2:
                        fn()
                    carry = [close1]
                    th1 = proj_thunks(b, 0, o["pat_attn"][0])
                    th2 = proj_thunks(b, 1, o["pat_attn"][1])
                    th3 = proj_thunks(b, 2, o["pat_attn"][2])
                    s2, s3 = o["shift2"], o["shift3"]
                    carry = attention_unit(b, 1, th1[:len(th1) - s2], carry)
                    carry = attention_unit(
                        b, 2, th1[len(th1) - s2:] + th2[:len(th2) - s3], carry)
                    carry = attention_unit(
                        b, 3, th2[len(th2) - s3:] + th3, carry,
                        reserve=o["reserve3"])
                # ---- tail: final qb3 close quarters interleaved with the
                # bf16 tail proj units (token tile q unblocks on quarter q)
                close1 = carry[0]
                tail_th = proj_thunks(B - 1, 3, o["pat_tail"],
                                      narrow_dma=False, tags=o["tail_tags"])
                for q in range(4):
                    close1(q)
                    for fn in tail_th[4 * q:4 * q + 4]:
                        fn()
                if debug:
                    for h in range(HPC):
                        nc.sync.dma_start(dbg["QT"][h], QTp[h][:])
                        nc.sync.dma_start(dbg["KT"][h], KTp[h][:])
                    nc.sync.dma_start(dbg["outH"][:, :, :], outH[:])
                    nc.sync.dma_start(dbg["outL"][:, :, :], outL[:])
                    for kt in range(NT):
                        nc.sync.dma_start(dbg["V"][kt], Vp[kt][:])

    nc.compile()
    return nc


def _split8(a):
    """fp8 e4m3 hi + residual lo (scale-free: lo rides on subnormals)."""
    f8 = mybir.dt.np(F8)
    hi = a.astype(f8)
    lo = (a - hi.astype(np.float32)).astype(f8)
    return hi, lo


def prep_inputs(x, w_qkv, w_o):
    """Host-side shard prep. Returns per-core input maps (fp8 hi/lo)."""
    bf = mybir.dt.np(BF16)
    x = np.asarray(x, dtype=np.float32).reshape(B * T, D)
    xT = np.ascontiguousarray(x.T).reshape(KC, P, B * T)
    # [2hl, KC, P, BT] -> [KP, P, 2hl, 2ch, BT]
    x8 = np.stack(_split8(xT)).reshape(2, KP, 2, P, B * T)
    x8 = np.ascontiguousarray(x8.transpose(1, 3, 0, 2, 4))
    w_qkv = np.asarray(w_qkv, dtype=np.float32) * WSCALE
    w_o = np.asarray(w_o, dtype=np.float32) * WSCALE

    tri = np.zeros((P, P), dtype=np.float32)
    kp = np.arange(P)[:, None]
    qu = np.arange(P)[None, :]
    tri[kp <= qu] = 1.0
    tri = tri.astype(bf)
    onr = np.full((1, P), 1.0, dtype=np.float32)

    in_maps = []
    for c in range(NCORES):
        h0, h1 = HPC * c, HPC * c + 1
        cols = []
        for h in (h0, h1):
            cols += [w_qkv[h * DK:(h + 1) * DK],            # Q rows
                     w_qkv[D + h * DK:D + (h + 1) * DK]]    # K rows
        # reorder to Q0 K0 Q1 K1 then V0 V1
        cols = [cols[0], cols[1], cols[2], cols[3],
                w_qkv[2 * D + h0 * DK:2 * D + (h0 + 1) * DK] * (VSCALE / WSCALE),
                w_qkv[2 * D + h1 * DK:2 * D + (h1 + 1) * DK] * (VSCALE / WSCALE)]
        w = np.ascontiguousarray(
            np.concatenate(cols, 0).T).reshape(KC, P, WC)
        w8 = np.stack(_split8(w)).reshape(2, KP, 2, P, WC)
        w8 = np.ascontiguousarray(w8.transpose(1, 3, 0, 2, 4))
        # wo: [dk, head, outcol] from w_o[:, core cols].T [256, D]
        woT = np.ascontiguousarray(
            w_o[:, HPC * DK * c:HPC * DK * (c + 1)].T)
        wo = np.ascontiguousarray(
            woT.reshape(HPC, DK, D).transpose(1, 0, 2))
        wo8 = np.ascontiguousarray(np.stack(_split8(wo), axis=1))
        wobf = np.ascontiguousarray(woT.reshape(HPC, DK, D)).astype(bf)
        in_maps.append({
            "x8": x8, "w8": w8, "wo8": wo8, "wobf": wobf,
            "tri": tri, "onr": onr,
        })
    return in_maps


_nc_cache = {}


def get_nc(debug=False, **opts):
    key = (debug, tuple(sorted((k, str(v)) for k, v in opts.items())))
    if key not in _nc_cache:
        _nc_cache[key] = build(debug=debug, **opts)
    return _nc_cache[key]


def run(x, w_qkv, w_o, debug=False, **opts):
    nc = get_nc(debug=debug, **opts)
    in_maps = prep_inputs(x, w_qkv, w_o)
    res = bass_utils.run_bass_kernel_spmd(nc, in_maps, core_ids=list(range(NCORES)))
    return res


def kernel(x, w_qkv, w_o):
    res = run(x, w_qkv, w_o)
    y = res.results[0]["y"].astype(np.float64)
    for c in range(1, NCORES):
        y += res.results[c]["y"]
    return (y / YDIV).astype(np.float32).reshape(B, T, D)


# revision 24
# speedup vs baseline: 1.1728x; 1.0139x over previous
"""Trainium2 Bass kernel for causal multi-head attention (dense transformer block).

Math (reference semantics):
    qkv = x @ w_qkv.T ; split into Q,K,V heads [B,H,T,dk]
    (rotary in the reference rotates Q and K of head h by a constant,
     time-independent orthogonal rotation R_h; since scores = (R_h q)·(R_h k)
     = q·k, the rotation cancels exactly and is skipped here)
    scores = causal_mask(Q @ K.T / sqrt(dk)); attn = softmax(scores)
    out = attn @ V ; y = out @ w_o.T

Sharding: head-parallel over 8 cores (2 heads/core, both batches).  Each core
computes a partial y (its heads' contribution through w_o columns); the host
sums the 8 partials (the "all-reduce").

v4 design (vs the bf16 v3 baseline):
  * QKV projection and out-projection run as 3-term compensated fp8
    (e4m3) DoubleRow matmuls: operands are split hi+lo (lo is the
    quantization residual, representable via fp8 subnormals without any
    rescale), and x@w = xh@wh + xl@wh + xh@wl accumulates in fp32 PSUM.
    DoubleRow contracts 2 k-tiles (256) per instruction at 0.5 cyc/row,
    so the three terms cost 0.75x the bf16 schedule on PE while being
    MORE accurate than bf16 (the residual kills the quantization error).
  * hi and lo planes are packed in one dram tensor so each chunk-pair /
    weight tile still costs ONE hwdge descriptor-generation slot (the
    dge fixed cost, not bytes, is the phase-1 DMA limiter).
  * Scale plumbing (no extra device ops): Q/K rows of w_qkv and all of
    w_o are scaled x64 host-side so their fp8 mantissas are in range
    (the exp scale absorbs the resulting x4096 on scores); V rows are
    scaled x32 so outT = ps_o * (1/denom) lands at x32 with good fp8
    hi/lo precision; y carries x32x64 and the host divides by 2048.
  * Attention (scores, exp, AV) stays bf16: plain-fp8 Q/K/ex/V all
    overshoot the error budget and compensated fp8 is slower than bf16
    at dk=128 contraction.
  * Softmax close-out: GpSimd partition_all_reduce folds the
    cross-partition denominator sum and its broadcast into one op, so
    DVE reciprocal directly yields the [P,NB] normalize multiplier (no
    PE ones-matmul); the normalized outT is written as fp8 hi + fp8
    residual lo feeding the DoubleRow out-projection.
  * DMA rule (hard-won): dma_start pairs elements by flat AP iteration
    order, NOT by dimension names.  Every dram tensor here is laid out
    so a DMA slice iterates exactly like its destination SBUF tile
    ([kp, p, hi/lo, parity, cols]); getting this wrong scrambles rows
    silently (the all-hi term still "works" because both operands get
    the same scramble -- only cross terms and the out-proj break).
  * Tail: the last batch's qb3 close-outs write bf16 outT directly
    (short chain, no fp8 split), quartered by token columns, and the
    final 16 proj units run bf16 2-matmul style interleaved with the
    quarters so PE restarts ~3us earlier.
  * Schedule (weaving of proj units into attention kt-pairs, deferred
    close-out chains, chunk-pair DMAs, causal narrowing with GpSimd
    memset/triangle masks) is inherited from v3.
"""

import contextlib

import numpy as np

import concourse.bacc as bacc
import concourse.bass as bass
import concourse.bass_isa as bass_isa
import concourse.mybir as mybir
import concourse.tile as tile
from concourse import bass_utils

B, T, D, H, DK = 2, 2048, 2048, 16, 128
NCORES = 8
HPC = H // NCORES  # heads per core
P = 128
NB = 512           # q-block / token-block / e-block width
KC = D // P        # 16 contraction chunks of the model dim
KP = KC // 2       # chunk pairs
QB = T // NB       # 4 q blocks per batch
NT = T // P        # 16 token tiles per batch
WC = 6 * P         # w columns per chunk: Q0 K0 Q1 K1 V0 V1
FP32 = mybir.dt.float32
F32R = mybir.dt.float32r
BF16 = mybir.dt.bfloat16
F8 = mybir.dt.float8e4
DR = mybir.MatmulPerfMode.DoubleRow
WSCALE = 64.0      # host scale on w_qkv Q/K rows and w_o (fp8 range)
VSCALE = 32.0      # host scale on w_qkv V rows => outT carries x32
YDIV = VSCALE * WSCALE  # host divides gathered y by this
SCALE = 1.0 / np.sqrt(DK)
ESCALE = SCALE / (WSCALE * WSCALE)  # exp() input scale: Q,K carry x64 each

DEFAULT_OPTS = dict(
    ex_bufs=6, xt_bufs=2, yt_bufs=4, s_bufs=2, o_bufs=2, y_bufs=2,
    loop_n=1,
    # yt-copy engine cycle per *hosting location* of the proj units
    pat_attn={0: "DAA", 1: "DA", 2: "DA", 3: "DAA"},
    pat_p1="DA", pat_tail="AD",
    defer_pairs=4,
    chain_pat={0: "DD", 1: "PD", 2: "PD", 3: "PD"},  # per qb (even, odd pair)
    mask_eng="D",
    qkv_q="A", qkv_k="D", qkv_v="DD", pipe=True, chain_bf16=True,
    oh_eng="D",     # outH fp8 copy engine
    tpr_eng="D",    # per-pair ex.lo+ex.hi add engine
    alrd=True,      # partition_all_reduce denominator (no broadcast stage)
    shift2=0,       # proj thunks moved from the qb1 weave into qb2's
    shift3=0,       # proj thunks moved from the qb2 weave into qb3's
    reserve3=6,     # fillers reserved past the final attention weave
    reserve=3,      # fillers reserved in non-final attention units
    tail_tags="YO",
)

_ENG_MAP = {"P": "gpsimd", "A": "scalar", "D": "vector"}


def _width(kt, qb):
    """Live q-suffix width of key tile kt within q-block qb (causal)."""
    j = kt - 4 * qb
    if j <= 0:
        return NB
    return NB - P * j


def build(debug=False, **opts):
    o = dict(DEFAULT_OPTS)
    o.update({k: v for k, v in opts.items() if k in DEFAULT_OPTS})
    nc = bacc.Bacc("TRN2", target_bir_lowering=False, debug=False,
                   num_devices=NCORES)
    # 3D dram layouts allow one DMA per chunk-pair / token tile; dim 0
    # packs the fp8 hi/lo planes so one DMA moves both.
    # dim order matches the SBUF tile iteration order exactly (dma_start
    # pairs elements by flat AP order): [kp, p, hi/lo, chunk-parity, cols]
    x8 = nc.dram_tensor("x8", [KP, P, 2, 2, B * T], F8, kind="ExternalInput")
    w8 = nc.dram_tensor("w8", [KP, P, 2, 2, WC], F8, kind="ExternalInput")
    wo8 = nc.dram_tensor("wo8", [P, 2, HPC, D], F8, kind="ExternalInput")
    wobf_d = nc.dram_tensor("wobf", [HPC, P, D], BF16, kind="ExternalInput")
    tri_d = nc.dram_tensor("tri", [P, P], BF16, kind="ExternalInput")
    onr_d = nc.dram_tensor("onr", [1, P], F32R, kind="ExternalInput")
    y = nc.dram_tensor("y", [B * T, D], BF16, kind="ExternalOutput")
    dbg = {}
    if debug:
        for nm in ("QT", "KT"):
            dbg[nm] = nc.dram_tensor(f"dbg_{nm}", [HPC, P, T], BF16,
                                     kind="ExternalOutput")
        dbg["outH"] = nc.dram_tensor("dbg_outH", [P, HPC, T], F8,
                                     kind="ExternalOutput")
        dbg["outL"] = nc.dram_tensor("dbg_outL", [P, HPC, T], F8,
                                     kind="ExternalOutput")
        dbg["V"] = nc.dram_tensor("dbg_V", [NT, P, 2 * P], BF16,
                                  kind="ExternalOutput")

    with tile.TileContext(nc) as tc:
        with (
            tc.tile_pool(name="const", bufs=1) as cpool,
            tc.tile_pool(name="xp", bufs=1) as xpool,
            tc.tile_pool(name="qkv", bufs=1) as qpool,
            tc.tile_pool(name="attn", bufs=1) as apool,
            tc.tile_pool(name="ps", bufs=1, space="PSUM") as pspool,
        ):
            # ---- constants / weights resident in SBUF ----
            # dim1 = hi/lo plane, dim2 = chunk parity (DoubleRow k-tiles)
            w_sb = [cpool.tile([P, 2, 2, WC], F8, name=f"w_{kp}")
                    for kp in range(KP)]
            wo_sb = cpool.tile([P, 2, HPC, D], F8, name="wo")
            wobf_sb = [cpool.tile([P, D], BF16, name=f"wobf{h}")
                       for h in range(HPC)]
            tri = cpool.tile([P, P], BF16, name="tri")
            onr = cpool.tile([1, P], F32R, name="onr")

            # persistent per-batch state (WAR deps recycle across batches)
            QTp = [qpool.tile([P, T], BF16, name=f"QT{h}") for h in range(HPC)]
            KTp = [qpool.tile([P, T], BF16, name=f"KT{h}") for h in range(HPC)]
            Vp = [qpool.tile([P, 2 * P], BF16, name=f"V{kt}") for kt in range(NT)]
            outH = qpool.tile([P, HPC, T], F8, name="outH")
            outL = qpool.tile([P, HPC, T], F8, name="outL")
            outTb = qpool.tile([P, HPC, NB], BF16, name="outTb")  # tail qb3

            def ps_tile(tag, shape, name, bufs):
                return pspool.tile(shape, FP32, name=name, tag=tag, bufs=bufs)

            def copy_on(code, dst, src):
                eng = _ENG_MAP[code]
                if eng == "gpsimd":
                    nc.gpsimd.tensor_copy(dst, src)
                elif eng == "scalar":
                    nc.scalar.copy(dst, src)
                else:
                    nc.vector.tensor_copy(dst, src)

            loop_ctx = (tc.For_i(0, o["loop_n"], 1, hint_engines=(
                            mybir.EngineType.PE, mybir.EngineType.Activation,
                            mybir.EngineType.DVE, mybir.EngineType.SP,
                            mybir.EngineType.Pool))
                        if o["loop_n"] > 1 else contextlib.nullcontext())

            if o["loop_n"] > 1:
                # weights/constants loaded once, outside the HW loop
                for kp in range(KP):
                    nc.sync.dma_start(w_sb[kp][:], w8[kp])
                nc.sync.dma_start(tri[:], tri_d[:, :])
                nc.sync.dma_start(onr[:], onr_d[:, :])
                nc.sync.dma_start(wo_sb[:], wo8[:, :, :, :])
                for h in range(HPC):
                    nc.sync.dma_start(wobf_sb[h][:], wobf_d[h])

            with loop_ctx:
                # ============ phase 1 generator (one token block) ============
                def load_nb(b, nb):
                    """Allocate + DMA the x tiles for token block (b, nb)."""
                    col0 = b * T + nb * NB
                    xt = [xpool.tile([P, 2, 2, NB], F8, name=f"x{kp}_{b}_{nb}",
                                     tag=f"x{kp}", bufs=o["xt_bufs"])
                          for kp in range(KP)]
                    first = b == 0 and nb == 0 and o["loop_n"] == 1
                    for kp in range(KP):
                        if first and kp == 0:
                            # split the first loads so the first Q/K matmuls
                            # unblock after the hi Q0K0 columns + hi x0
                            nc.sync.dma_start(w_sb[0][:, 0, :, 0:2 * P],
                                              w8[0, :, 0, :, 0:2 * P])
                            nc.sync.dma_start(
                                xt[0][:, 0],
                                x8[0, :, 0, :, col0:col0 + NB])
                            nc.sync.dma_start(w_sb[0][:, 0, :, 2 * P:WC],
                                              w8[0, :, 0, :, 2 * P:WC])
                            nc.sync.dma_start(
                                xt[0][:, 1],
                                x8[0, :, 1, :, col0:col0 + NB])
                            nc.sync.dma_start(w_sb[0][:, 1], w8[0, :, 1])
                            continue
                        if first:
                            nc.sync.dma_start(w_sb[kp][:], w8[kp])
                        nc.sync.dma_start(
                            xt[kp][:], x8[kp, :, :, :, col0:col0 + NB])
                        if first and kp == 1:
                            nc.sync.dma_start(tri[:], tri_d[:, :])
                            nc.sync.dma_start(onr[:], onr_d[:, :])
                    return xt

                # (w_hl, x_hl) term order: hi@hi, hi-w@lo-x, lo-w@hi-x
                TERMS = ((0, 0), (0, 1), (1, 0))

                def phase1_nb(b, nb, xt, mid=None):
                    """QKV projection for token block (b, nb), k-outer,
                    3-term compensated fp8 DoubleRow.  Yields after each
                    chunk-pair (8) + drains; `mid` thunk (next-block
                    prefetch) fires after chunk-pair 5."""
                    if b == 0 and nb == 1 and o["loop_n"] == 1:
                        nc.sync.dma_start(wo_sb[:], wo8[:, :, :, :])
                    if b == 1 and nb == 1 and o["loop_n"] == 1:
                        for h in range(HPC):
                            nc.sync.dma_start(wobf_sb[h][:], wobf_d[h])

                    S0 = ps_tile("S", [P, 2, NB], f"p1s0_{b}_{nb}", o["s_bufs"])
                    S1 = ps_tile("S", [P, 2, NB], f"p1s1_{b}_{nb}", o["s_bufs"])
                    # V token-tile groups need a PSUM bank each (one
                    # accumulation group per bank): two sub-sweeps of 2.
                    V01 = [ps_tile("O", [P, NB], f"p1v{t}_{b}_{nb}", o["o_bufs"])
                           for t in range(2)]
                    SQK = (S0[:, 0, :], S0[:, 1, :], S1[:, 0, :], S1[:, 1, :])
                    first_nb = b == 0 and nb == 0 and o["loop_n"] == 1
                    for kp in range(KP):
                        for ti, (wi, xi) in enumerate(TERMS):
                            st = kp == 0 and ti == 0
                            sp = kp == KP - 1 and ti == 2
                            for m in range(4):
                                nc.tensor.matmul(
                                    SQK[m],
                                    w_sb[kp][:, wi, :, m * P:(m + 1) * P],
                                    xt[kp][:, xi], start=st, stop=sp,
                                    perf_mode=DR)
                            for t in range(2):
                                nc.tensor.matmul(
                                    V01[t][:, 0:2 * P],
                                    xt[kp][:, xi, :, t * P:(t + 1) * P],
                                    w_sb[kp][:, wi, :, 4 * P:6 * P],
                                    start=st, stop=sp, perf_mode=DR)
                        if kp == 5 and mid is not None:
                            mid()
                        yield
                    # drain V first (V2/V3 sweeps wait on these PSUM
                    # banks), then QK (next block's S tiles are far off)
                    csl = slice(nb * NB, (nb + 1) * NB)
                    for t in range(2):
                        copy_on(o["qkv_v"][t % len(o["qkv_v"])],
                                Vp[nb * 4 + t][:], V01[t][:, 0:2 * P])
                    copy_on(o["qkv_q"], QTp[0][:, csl], S0[:, 0, :])
                    copy_on(o["qkv_k"], KTp[0][:, csl], S0[:, 1, :])
                    copy_on(o["qkv_q"], QTp[1][:, csl], S1[:, 0, :])
                    copy_on(o["qkv_k"], KTp[1][:, csl], S1[:, 1, :])
                    yield
                    # V2 then V3 sequentially: each holds only ONE O slot,
                    # so attention(qb0) can interleave using the other slot
                    for t in range(2, 4):
                        Vt = ps_tile("Y", [P, NB], f"p1v{t}_{b}_{nb}",
                                     o["y_bufs"])
                        for kp in range(KP):
                            for ti, (wi, xi) in enumerate(TERMS):
                                nc.tensor.matmul(
                                    Vt[:, 0:2 * P],
                                    xt[kp][:, xi, :, t * P:(t + 1) * P],
                                    w_sb[kp][:, wi, :, 4 * P:6 * P],
                                    start=(kp == 0 and ti == 0),
                                    stop=(kp == KP - 1 and ti == 2),
                                    perf_mode=DR)
                            if kp % 2 == 1:
                                yield
                        copy_on(o["qkv_v"][t % len(o["qkv_v"])],
                                Vp[nb * 4 + t][:], Vt[:, 0:2 * P])
                        yield

                # ============ attention generator (one head) ============
                def attention_gen(b, h, qb):
                    """Yields once per kt-pair.  Returns the deferred
                    close-out thunk (broadcast + normalize + fp8 split)."""
                    nkt = 4 * qb + 4
                    qsl0 = qb * NB
                    ps_o = ps_tile("O", [P, NB], f"pso_{b}_{h}_{qb}", o["o_bufs"])
                    # two running-sum chains: even pairs / odd pairs
                    cdt = BF16 if o["chain_bf16"] else FP32
                    exs = [apool.tile([P, NB], cdt, name=f"exs{i}_{b}_{h}_{qb}",
                                      tag=f"exsum{i}", bufs=2) for i in range(2)]
                    npair = nkt // 2

                    def emit_scores_exp(p):
                        a, c = 2 * p, 2 * p + 1
                        oa, oc = NB - _width(a, qb), NB - _width(c, qb)
                        ps_s = ps_tile("S", [P, 2, NB], f"pss_{b}_{h}_{qb}_{p}",
                                       o["s_bufs"])
                        nc.tensor.matmul(ps_s[:, 0, oa:NB],
                                         KTp[h][:, a * P:(a + 1) * P],
                                         QTp[h][:, qsl0 + oa:qsl0 + NB],
                                         start=True, stop=True)
                        nc.tensor.matmul(ps_s[:, 1, oc:NB],
                                         KTp[h][:, c * P:(c + 1) * P],
                                         QTp[h][:, qsl0 + oc:qsl0 + NB],
                                         start=True, stop=True)
                        ex = apool.tile([P, 2, NB], BF16,
                                        name=f"ex_{b}_{h}_{qb}_{p}",
                                        tag="ex", bufs=o["ex_bufs"])
                        if oa == oc:
                            nc.scalar.activation(ex[:, :, oa:NB],
                                                 ps_s[:, :, oa:NB],
                                                 mybir.ActivationFunctionType.Exp,
                                                 scale=ESCALE)
                        else:
                            nc.scalar.activation(ex[:, 0, oa:NB],
                                                 ps_s[:, 0, oa:NB],
                                                 mybir.ActivationFunctionType.Exp,
                                                 scale=ESCALE)
                            nc.scalar.activation(ex[:, 1, oc:NB],
                                                 ps_s[:, 1, oc:NB],
                                                 mybir.ActivationFunctionType.Exp,
                                                 scale=ESCALE)
                        return (p, ex, oa, oc)

                    def emit_post(st):
                        p, ex, oa, oc = st
                        a, c = 2 * p, 2 * p + 1
                        # zero dead prefixes of narrowed (diagonal) tiles
                        if oa > 0:
                            nc.gpsimd.memset(ex[:, 0, 0:oa], 0.0)
                        if oc > 0:
                            nc.gpsimd.memset(ex[:, 1, 0:oc], 0.0)
                        # triangle masks on diagonal tiles
                        for half, kt, off in ((0, a, oa), (1, c, oc)):
                            if kt >= 4 * qb:
                                sl = ex[:, half, off:off + P]
                                if o["mask_eng"] == "P":
                                    nc.gpsimd.tensor_mul(sl, sl, tri[:])
                                else:
                                    nc.vector.tensor_mul(sl, sl, tri[:])
                        # denominator partial: exs[p%2] += ex.lo + ex.hi
                        tpr = apool.tile([P, NB], BF16,
                                         name=f"tp_{b}_{h}_{qb}_{p}",
                                         tag="tpr", bufs=2)
                        getattr(nc, _ENG_MAP[o["tpr_eng"]]).tensor_add(
                            tpr[:], ex[:, 0, :], ex[:, 1, :])
                        cp = o["chain_pat"][qb] if isinstance(o["chain_pat"], dict) else o["chain_pat"]
                        eng = getattr(nc, _ENG_MAP[cp[p % 2]])
                        if p < 2:
                            eng.tensor_copy(exs[p % 2][:], tpr[:])
                        else:
                            eng.tensor_add(exs[p % 2][:], exs[p % 2][:], tpr[:])
                        # AV accumulation
                        nc.tensor.matmul(ps_o[:, oa:NB],
                                         Vp[a][:, h * P:(h + 1) * P],
                                         ex[:, 0, oa:NB],
                                         start=(p == 0), stop=False,
                                         skip_group_check=True)
                        nc.tensor.matmul(ps_o[:, oc:NB],
                                         Vp[c][:, h * P:(h + 1) * P],
                                         ex[:, 1, oc:NB],
                                         start=False, stop=(p == npair - 1),
                                         skip_group_check=True)

                    # denominator tiles (hoisted: the fin path emits its
                    # quarter chains inside the pair loop)
                    mrg = apool.tile([P, NB], BF16, name=f"mg_{b}_{h}_{qb}",
                                     tag="mrg", bufs=2)
                    # final-batch qb3: both heads take the short bf16 tail
                    finq = b == B - 1 and qb == QB - 1
                    fin = finq and h == 1
                    qsls = ([slice(q * P, (q + 1) * P) for q in range(4)]
                            if fin else [slice(0, NB)])
                    if o["alrd"]:
                        dnb = apool.tile([P, NB], FP32, name=f"dn_{b}_{h}_{qb}",
                                         tag="dn", bufs=2)
                        rec = apool.tile([P, NB], F32R,
                                         name=f"rec_{b}_{h}_{qb}",
                                         tag="rec", bufs=2)

                        def emit_chain_qs(qs):
                            with nc.allow_low_precision(reason="f32r recip"):
                                for q in qs:
                                    cs = slice(q * P, (q + 1) * P)
                                    nc.vector.tensor_add(mrg[:, cs],
                                                         exs[0][:, cs],
                                                         exs[1][:, cs])
                                    nc.gpsimd.partition_all_reduce(
                                        dnb[:, cs], mrg[:, cs], channels=P,
                                        reduce_op=bass_isa.ReduceOp.add)
                                    nc.vector.reciprocal(rec[:, cs],
                                                         dnb[:, cs])
                        fin_chain = [emit_chain_qs]
                    else:
                        fin_chain = [None]

                    # software pipeline: scores/exp of p+1 before AV of p
                    if o["pipe"]:
                        st = emit_scores_exp(0)
                        for p in range(npair):
                            nxt = (emit_scores_exp(p + 1)
                                   if p + 1 < npair else None)
                            emit_post(st)
                            if fin and p == npair - 2:
                                # the last pair is dead in columns [0:2P):
                                # quarters q0/q1 of the denominator AND the
                                # outTb normalize close one pair early (exs
                                # chains and ps_o are complete there), so
                                # tail tiles tt12/tt13 unblock before the
                                # last pair retires
                                fin_chain[0]([0, 1])
                                for q in (0, 1):
                                    cq = slice(q * P, (q + 1) * P)
                                    nc.vector.tensor_mul(outTb[:, h, cq],
                                                         ps_o[:, cq],
                                                         rec[:, cq])
                            if fin and p == npair - 1:
                                fin_chain[0]([2, 3])
                                for q in (2, 3):
                                    cq = slice(q * P, (q + 1) * P)
                                    nc.vector.tensor_mul(outTb[:, h, cq],
                                                         ps_o[:, cq],
                                                         rec[:, cq])
                            st = nxt
                            yield
                    else:
                        for p in range(npair):
                            emit_post(emit_scores_exp(p))
                            yield
                    # denominator: merge chains, cross-partition reduce,
                    # recip; alrd=True folds reduce+broadcast into one
                    # gpsimd partition_all_reduce
                    if o["alrd"]:
                        if not fin:
                            with nc.allow_low_precision(reason="f32r recip"):
                                for cs in qsls:
                                    nc.vector.tensor_add(mrg[:, cs],
                                                         exs[0][:, cs],
                                                         exs[1][:, cs])
                                    nc.gpsimd.partition_all_reduce(
                                        dnb[:, cs], mrg[:, cs], channels=P,
                                        reduce_op=bass_isa.ReduceOp.add)
                                    nc.vector.reciprocal(rec[:, cs],
                                                         dnb[:, cs])
                    else:
                        dn = apool.tile([1, NB], FP32, name=f"dn_{b}_{h}_{qb}",
                                        tag="dn", bufs=2)
                        rc1 = apool.tile([1, NB], F32R,
                                         name=f"rc_{b}_{h}_{qb}",
                                         tag="rc1", bufs=2)
                        with nc.allow_low_precision(reason="f32r recip"):
                            for cs in qsls:
                                nc.vector.tensor_add(mrg[:, cs],
                                                     exs[0][:, cs],
                                                     exs[1][:, cs])
                                nc.gpsimd.tensor_reduce(
                                    dn[:1, cs], mrg[:, cs],
                                    axis=mybir.AxisListType.C,
                                    op=mybir.AluOpType.add)
                                nc.vector.reciprocal(rc1[:1, cs],
                                                     dn[:1, cs])

                    def bc_of(cs):
                        if o["alrd"]:
                            return rec[:, cs]
                        bc = apool.tile([P, NB], F32R, name=f"bc_{b}_{h}_{qb}",
                                        tag="bc", bufs=2)
                        nc.gpsimd.partition_broadcast(bc[:, cs], rc1[:1, cs])
                        return bc[:, cs]

                    def close(q=None):
                        if fin and o["alrd"]:
                            return  # emitted early inside the pair loop
                        if finq:
                            # bf16 tail: outTb = ps_o * bc, no fp8 split
                            slices = ((slice(0, NB),) if q is None
                                      else (slice(q * P, (q + 1) * P),))
                            for cs in slices:
                                nc.vector.tensor_mul(outTb[:, h, cs],
                                                     ps_o[:, cs], bc_of(cs))
                            return
                        t1 = apool.tile([P, NB], BF16, name=f"t1_{b}_{h}_{qb}",
                                        tag="t1", bufs=2)
                        cs = slice(0, NB)
                        osl = slice(qsl0, qsl0 + NB)
                        nc.vector.tensor_mul(t1[:, cs], ps_o[:, cs], bc_of(cs))
                        copy_on(o["oh_eng"], outH[:, h, osl], t1[:, cs])
                        nc.vector.tensor_sub(outL[:, h, osl], t1[:, cs],
                                             outH[:, h, osl])
                    return close

                # ============ proj units ============
                yts = {}

                def proj_unit(b, tt, eb, eng_code, narrow_dma=False,
                              tag="Y"):
                    ps_y = ps_tile(tag, [P, NB], f"psy_{b}_{tt}_{eb}",
                                   o["y_bufs"])
                    tsl = slice(tt * P, (tt + 1) * P)
                    esl = slice(eb * NB, (eb + 1) * NB)
                    if b == B - 1 and tt >= 12:
                        # bf16 tail path (outTb holds qb3 tokens)
                        bsl = slice((tt - 12) * P, (tt - 11) * P)
                        nc.tensor.matmul(ps_y[:], outTb[:, 0, bsl],
                                         wobf_sb[0][:, esl],
                                         start=True, stop=False)
                        nc.tensor.matmul(ps_y[:], outTb[:, 1, bsl],
                                         wobf_sb[1][:, esl],
                                         start=False, stop=True)
                    else:
                        nc.tensor.matmul(ps_y[:], outH[:, :, tsl],
                                         wo_sb[:, 0, :, esl],
                                         start=True, stop=False, perf_mode=DR)
                        nc.tensor.matmul(ps_y[:], outL[:, :, tsl],
                                         wo_sb[:, 0, :, esl],
                                         start=False, stop=False, perf_mode=DR)
                        nc.tensor.matmul(ps_y[:], outH[:, :, tsl],
                                         wo_sb[:, 1, :, esl],
                                         start=False, stop=True, perf_mode=DR)
                    if eb == 0:
                        yts[(b, tt)] = apool.tile([P, QB, NB], BF16,
                                                  name=f"yt_{b}_{tt}",
                                                  tag="yt", bufs=o["yt_bufs"])
                    yt = yts[(b, tt)]
                    copy_on(eng_code, yt[:, eb, :], ps_y[:])
                    if narrow_dma:
                        # tail: stream each e-block out as soon as copied
                        nc.sync.dma_start(
                            y[b * T + tt * P:b * T + (tt + 1) * P,
                              eb * NB:(eb + 1) * NB], yt[:, eb, :])
                    elif b == B - 1 and tt >= 12 and eb in (1, QB - 1):
                        # final tiles: stream column-halves so the last DMA
                        # after the last matmul is half-sized
                        hsl = slice(0, 2) if eb == 1 else slice(2, QB)
                        nc.sync.dma_start(
                            y[b * T + tt * P:b * T + (tt + 1) * P,
                              hsl.start * NB:hsl.stop * NB], yt[:, hsl, :])
                    elif eb == QB - 1 and not (b == B - 1 and tt >= 12):
                        nc.sync.dma_start(
                            y[b * T + tt * P:b * T + (tt + 1) * P, :], yt[:])

                def proj_thunks(b, pqb, pat, narrow_dma=False, tags="Y"):
                    th = []
                    i = 0
                    for tt in range(4 * pqb, 4 * pqb + 4):
                        for eb in range(QB):
                            code = pat[i % len(pat)]
                            tag = tags[i % len(tags)]
                            th.append(lambda b=b, tt=tt, eb=eb, code=code,
                                      tag=tag:
                                      proj_unit(b, tt, eb, code, narrow_dma,
                                                tag))
                            i += 1
                    return th

                # ============ weaving driver ============
                def weave(gen, fillers, carry, defer=None):
                    """Run gen; after each yield emit carry thunks (once,
                    after o['defer_pairs'] yields) and a fair share of
                    fillers (popped from the shared list)."""
                    n = 0
                    held = 0
                    try:
                        while True:
                            next(gen)
                            n += 1
                            if n >= o["defer_pairs"] and carry:
                                for fn in carry:
                                    fn()
                                carry = []
                            if defer:
                                held += defer.pop(0)
                            if not carry:
                                while held > 0 and fillers:
                                    fillers.pop(0)()
                                    held -= 1
                    except StopIteration as si:
                        for fn in carry:
                            fn()
                        return si.value

                def share(nfill, nsteps):
                    base, rem = divmod(nfill, nsteps)
                    return [base + (1 if i < rem else 0) for i in range(nsteps)]

                def attention_unit(b, qb, fillers, carry, reserve=None,
                                   final=False):
                    if reserve is None:
                        reserve = o["reserve"]
                    npair = 2 * qb + 2
                    # hold a few fillers back to cover the close-out chain
                    # latency after the last AV pair
                    nres = min(reserve, len(fillers))
                    sh = share(len(fillers) - nres, 2 * npair)
                    close0 = weave(attention_gen(b, 0, qb), fillers, carry,
                                   defer=sh[:npair])
                    close1 = weave(attention_gen(b, 1, qb), fillers, [close0],
                                   defer=sh[npair:])
                    for fn in fillers:  # reserved + leftovers
                        fn()
                    del fillers[:]
                    return [close1]

                def prefix(gen, n):
                    for _ in range(n):
                        next(gen)
                        yield

                # ============ main schedule ============
                carry = []
                xts = {}
                for b in range(B):
                    # ---- phase 1 (+ second half of prev batch qb3 proj) ----
                    if b > 0:
                        ph1_fill = proj_thunks(b - 1, 3, o["pat_p1"])[8:]
                        qb0_fill = proj_thunks(b - 1, 3, o["pat_attn"][3])[:8]
                    else:
                        ph1_fill, qb0_fill = [], []
                    if b == 0:
                        xts["cur"] = load_nb(0, 0)
                    vtails = []
                    for nb in range(QB - 1):
                        nxt = [b, nb + 1]
                        mid = (lambda nxt=nxt:
                               xts.__setitem__("next", load_nb(*nxt)))
                        g_nb = phase1_nb(b, nb, xts["cur"], mid)
                        fills = vtails + ph1_fill
                        nf = len(fills)
                        weave(prefix(g_nb, 9), fills, carry,
                              defer=share(nf, 9) if nf else None)
                        for fn in fills:
                            fn()
                        del ph1_fill[:len(ph1_fill)]
                        # this block's V2/V3 tail fills the next block
                        vtails = [(lambda g=g_nb: next(g, None))
                                  for _ in range(10)]
                        xts["cur"] = xts["next"]
                        carry = []
                    # nb3: QK sweep + V01; the V2/V3 tail becomes PE filler
                    # for the attention(qb0) unit
                    mid = ((lambda: xts.__setitem__("next", load_nb(b + 1, 0)))
                           if b + 1 < B else None)
                    g_ph = phase1_nb(b, 3, xts["cur"], mid)
                    fills3 = vtails + ph1_fill
                    weave(prefix(g_ph, 9), fills3, carry,
                          defer=share(len(fills3), 9) if fills3 else None)
                    for fn in fills3:
                        fn()
                    if b + 1 < B:
                        xts["cur"] = xts["next"]
                    carry = []
                    tails = [(lambda: next(g_ph, None)) for _ in range(10)]
                    # ---- qb0 unit merged with phase-1 V tail ----
                    # one V2 step first fixes the O-slot rotation so the V3
                    # sweep can safely cover the h0 close-out chain
                    tails.pop(0)()
                    close0 = weave(attention_gen(b, 0, 0), tails, carry,
                                   defer=[2, 2])
                    for _ in range(3):  # V3 progress covers the dn/recip chain
                        if tails:
                            tails.pop(0)()
                    close0()
                    fill2 = tails + qb0_fill
                    close1 = weave(attention_gen(b, 1, 0), fill2,
                                   [], defer=[2, 2])
                    for fn in fill2:
                        fn()
                    carry = [close1]
                    th1 = proj_thunks(b, 0, o["pat_attn"][0])
                    th2 = proj_thunks(b, 1, o["pat_attn"][1])
                    th3 = proj_thunks(b, 2, o["pat_attn"][2])
                    s2, s3 = o["shift2"], o["shift3"]
                    carry = attention_unit(b, 1, th1[:len(th1) - s2], carry)
                    carry = attention_unit(
                        b, 2, th1[len(th1) - s2:] + th2[:len(th2) - s3], carry)
                    carry = attention_unit(
                        b, 3, th2[len(th2) - s3:] + th3, carry,
                        reserve=o["reserve3"])
                # ---- tail: final qb3 close quarters interleaved with the
                # bf16 tail proj units (token tile q unblocks on quarter q)
                close1 = carry[0]
                tail_th = proj_thunks(B - 1, 3, o["pat_tail"],
                                      narrow_dma=False, tags=o["tail_tags"])
                for q in range(4):
                    close1(q)
                    for fn in tail_th[4 * q:4 * q + 4]:
                        fn()
                if debug:
                    for h in range(HPC):
                        nc.sync.dma_start(dbg["QT"][h], QTp[h][:])
                        nc.sync.dma_start(dbg["KT"][h], KTp[h][:])
                    nc.sync.dma_start(dbg["outH"][:, :, :], outH[:])
                    nc.sync.dma_start(dbg["outL"][:, :, :], outL[:])
                    for kt in range(NT):
                        nc.sync.dma_start(dbg["V"][kt], Vp[kt][:])

    nc.compile()
    return nc


def _split8(a):
    """fp8 e4m3 hi + residual lo (scale-free: lo rides on subnormals)."""
    f8 = mybir.dt.np(F8)
    hi = a.astype(f8)
    lo = (a - hi.astype(np.float32)).astype(f8)
    return hi, lo


def prep_inputs(x, w_qkv, w_o):
    """Host-side shard prep. Returns per-core input maps (fp8 hi/lo)."""
    bf = mybir.dt.np(BF16)
    x = np.asarray(x, dtype=np.float32).reshape(B * T, D)
    xT = np.ascontiguousarray(x.T).reshape(KC, P, B * T)
    # [2hl, KC, P, BT] -> [KP, P, 2hl, 2ch, BT]
    x8 = np.stack(_split8(xT)).reshape(2, KP, 2, P, B * T)
    x8 = np.ascontiguousarray(x8.transpose(1, 3, 0, 2, 4))
    w_qkv = np.asarray(w_qkv, dtype=np.float32) * WSCALE
    w_o = np.asarray(w_o, dtype=np.float32) * WSCALE

    tri = np.zeros((P, P), dtype=np.float32)
    kp = np.arange(P)[:, None]
    qu = np.arange(P)[None, :]
    tri[kp <= qu] = 1.0
    tri = tri.astype(bf)
    onr = np.full((1, P), 1.0, dtype=np.float32)

    in_maps = []
    for c in range(NCORES):
        h0, h1 = HPC * c, HPC * c + 1
        cols = []
        for h in (h0, h1):
            cols += [w_qkv[h * DK:(h + 1) * DK],            # Q rows
                     w_qkv[D + h * DK:D + (h + 1) * DK]]    # K rows
        # reorder to Q0 K0 Q1 K1 then V0 V1
        cols = [cols[0], cols[1], cols[2], cols[3],
                w_qkv[2 * D + h0 * DK:2 * D + (h0 + 1) * DK] * (VSCALE / WSCALE),
                w_qkv[2 * D + h1 * DK:2 * D + (h1 + 1) * DK] * (VSCALE / WSCALE)]
        w = np.ascontiguousarray(
            np.concatenate(cols, 0).T).reshape(KC, P, WC)
        w8 = np.stack(_split8(w)).reshape(2, KP, 2, P, WC)
        w8 = np.ascontiguousarray(w8.transpose(1, 3, 0, 2, 4))
        # wo: [dk, head, outcol] from w_o[:, core cols].T [256, D]
        woT = np.ascontiguousarray(
            w_o[:, HPC * DK * c:HPC * DK * (c + 1)].T)
        wo = np.ascontiguousarray(
            woT.reshape(HPC, DK, D).transpose(1, 0, 2))
        wo8 = np.ascontiguousarray(np.stack(_split8(wo), axis=1))
        wobf = np.ascontiguousarray(woT.reshape(HPC, DK, D)).astype(bf)
        in_maps.append({
            "x8": x8, "w8": w8, "wo8": wo8, "wobf": wobf,
            "tri": tri, "onr": onr,
        })
    return in_maps


_nc_cache = {}


def get_nc(debug=False, **opts):
    key = (debug, tuple(sorted((k, str(v)) for k, v in opts.items())))
    if key not in _nc_cache:
        _nc_cache[key] = build(debug=debug, **opts)
    return _nc_cache[key]


def run(x, w_qkv, w_o, debug=False, **opts):
    nc = get_nc(debug=debug, **opts)
    in_maps = prep_inputs(x, w_qkv, w_o)
    res = bass_utils.run_bass_kernel_spmd(nc, in_maps, core_ids=list(range(NCORES)))
    return res


def kernel(x, w_qkv, w_o):
    res = run(x, w_qkv, w_o)
    y = res.results[0]["y"].astype(np.float64)
    for c in range(1, NCORES):
        y += res.results[c]["y"]
    return (y / YDIV).astype(np.float32).reshape(B, T, D)
